# revision 1
# baseline (speedup 1.0000x reference)
"""Causal multi-head attention (RoPE) on 8 TRN2 NeuronCores.

Problem: x[2,2048,2048] -> qkv proj -> rope -> causal attention (16 heads,
head_dim 128) -> output proj + bias. Sharding: (batch, head-group) across the
8 cores - core c handles batch c//4 and heads 4*(c%4)..4*(c%4)+3. Each core
computes a partial output projection over its heads' channels; the host sums
the 4 partials per batch and adds b_o.

Fully SBUF-resident pipeline, no DRAM scratch roundtrips. The QKV
contraction runs in 2 passes of 8 c-tiles each (9 rotating x^T slots);
pass-0 partials evict via ACT copy; pass-1 q/k partials merge back into
PSUM with an identity matmul (keeps the DVE stream clear for the rope
chains) and evict via ACT, while v partials merge with a DVE add.
RoPE is applied in place (partition half-swap via SBUF->SBUF DMAs, sign
folded into sinT host-side), with chains emitted inside the attention jb
loop so the per-engine instruction streams interleave. q^T,k^T (all 4
heads) and batched v stay in SBUF through attention; ctx stays in SBUF
through the output projection, which runs fused per 512-token block.

All matmuls keep K (contraction) on partitions:
  - q,k produced transposed [d, tok]; v produced natural [tok, (h,d)]
  - scores computed transposed s^T[tk, tq] (lhsT=k^T tile, rhs=q^T block):
    softmax needs no transposes; exp on ACT; row-sums on DVE (lacc),
    partition-reduced and broadcast back via tiny ones-matmuls
  - AV: ctx^T[d, tq] = v.T @ p^T with PSUM accumulation over tk tiles
  - outproj: y[tok, o] accumulates the 4 heads' ctx^T.T @ W_o^T slices
Matmuls run in float32r (1 cycle/row at N>=256 vs 4 for fp32).
"""
import math

import numpy as np

import concourse.bacc as bacc
import concourse.mybir as mybir
import concourse.tile as tile
from concourse.bass_utils import run_bass_kernel_spmd

P = 128           # partitions / head_dim
T = 2048          # context length
C = 2048          # d_model
NKT = C // P      # 16 contraction tiles
NTT = T // P      # 16 token tiles
NB = T // 512     # 4 query blocks of 512
HPC = 4           # heads per core
NPASS = 2         # contraction passes
KPP = NKT // NPASS  # c-tiles per pass
NCORES = 8
SCALE = 1.0 / math.sqrt(P)
MASK_NEG = -1.0e30

F32 = mybir.dt.float32
F32R = mybir.dt.float32r
EXP = mybir.ActivationFunctionType.Exp
MULT = mybir.AluOpType.mult
ADD = mybir.AluOpType.add

_CACHE = {}


def _build(matmul_dt=F32R):
    nc = bacc.Bacc("TRN2", target_bir_lowering=False, debug=False,
                   num_devices=NCORES)
    dt = matmul_dt
    xT = nc.dram_tensor("xT", (C, T), dt, kind="ExternalInput").ap()
    wq = nc.dram_tensor("wq", (HPC, P, NKT, P), dt, kind="ExternalInput").ap()
    wk = nc.dram_tensor("wk", (HPC, P, NKT, P), dt, kind="ExternalInput").ap()
    wv = nc.dram_tensor("wv", (NKT, P, HPC * P), dt, kind="ExternalInput").ap()
    wo = nc.dram_tensor("wo", (HPC, P, C), dt, kind="ExternalInput").ap()
    cosT = nc.dram_tensor("cosT", (P, T), F32, kind="ExternalInput").ap()
    sinT = nc.dram_tensor("sinT", (P, T), F32, kind="ExternalInput").ap()
    tri = nc.dram_tensor("tri", (P, P), F32, kind="ExternalInput").ap()
    m3 = nc.dram_tensor("m3", (P, 2 * P), F32, kind="ExternalInput").ap()
    ones = nc.dram_tensor("ones", (P, P), dt, kind="ExternalInput").ap()
    eye = nc.dram_tensor("eye", (P, P), dt, kind="ExternalInput").ap()
    y = nc.dram_tensor("y", (T, C), F32, kind="ExternalOutput").ap()

    with tile.TileContext(nc) as tc:
        with (
            tc.tile_pool(name="gconst", bufs=1) as gpool,
            tc.tile_pool(name="qkbuf", bufs=1) as qkpool,
            tc.tile_pool(name="vbuf", bufs=1) as vpool,
        ):
            tri_sb = gpool.tile([P, P], F32, tag="tri")
            m3_sb = gpool.tile([P, 2 * P], F32, tag="m3")
            ones_sb = gpool.tile([P, P], dt, tag="ones")
            eye_sb = gpool.tile([P, P], dt, tag="eye")

            # persistent activations (SBUF-resident across phases)
            qk_sb = {}
            for h in range(HPC):
                for part in ("q", "k"):
                    for nb in range(NB):
                        t_ = qkpool.tile([P, 512], dt, tag=f"{part}{h}n{nb}",
                                         name=f"{part}{h}n{nb}_sb")
                        qk_sb[(part, h, nb)] = t_
            v_sb = [vpool.tile([P, HPC * P], dt, tag=f"vb{i}", name=f"v{i}_sb")
                    for i in range(NTT)]

            # ------------- Phase 1: QKV projection (4 passes) + rope --------
            with (
                tc.tile_pool(name="xp", bufs=1) as xpool,
                tc.tile_pool(name="wp", bufs=2) as wpool,
                tc.tile_pool(name="psv", bufs=1, space="PSUM") as psv,
                tc.tile_pool(name="ps1", bufs=2, space="PSUM") as ps1,
            ):
                xt_pref = {}
                for ps in range(NPASS):
                    if ps == 1:
                        nc.sync.dma_start(eye_sb[:], eye)
                        nc.sync.dma_start(tri_sb[:], tri)
                        nc.sync.dma_start(m3_sb[:], m3)
                        nc.sync.dma_start(ones_sb[:], ones)
                    kts = [ps * KPP + j for j in range(KPP)]
                    xt = {}
                    wvt = {}
                    w_tiles = {}

                    def load_w(h, part, wtens):
                        tiles = []
                        for half in range(2):
                            wt = wpool.tile([P, KPP // 2, P], dt, tag="w",
                                            bufs=4, name=f"w_{part}{h}_{half}")
                            nc.sync.dma_start(
                                wt[:], wtens[h][:, ps * KPP + half * (KPP // 2):
                                                ps * KPP + (half + 1) * (KPP // 2), :])
                            tiles.append(wt)
                        w_tiles[(part, h)] = tiles

                    for kt in kts:
                        if kt in xt_pref:
                            xt[kt] = xt_pref.pop(kt)
                        else:
                            x_ = xpool.tile([P, T], dt, tag=f"x{kt % 9}",
                                            bufs=1, name=f"x_{kt}")
                            nc.sync.dma_start(x_[:], xT[kt * P:(kt + 1) * P, :])
                            xt[kt] = x_
                        wv_ = wpool.tile([P, HPC * P], dt, tag=f"wv{kt % KPP}",
                                         bufs=1, name=f"wv_{kt}")
                        nc.sync.dma_start(wv_[:], wv[kt])
                        wvt[kt] = wv_
                        if kt == kts[1]:
                            load_w(0, "q", wq)
                        elif kt == kts[3]:
                            load_w(0, "k", wk)
                        elif ps == 0 and kt == kts[-1]:
                            # prefetch pass-1's first x tile into the spare slot
                            pk = KPP
                            px = xpool.tile([P, T], dt, tag=f"x{pk % 9}",
                                            bufs=1, name=f"x_{pk}")
                            nc.sync.dma_start(px[:], xT[pk * P:(pk + 1) * P, :])
                            xt_pref[pk] = px

                    # v: 4 tok-groups of 4 PSUM banks
                    for vg in range(4):
                        vaccs = []
                        for j, kt in enumerate(kts):
                            for i in range(4):
                                tt = vg * 4 + i
                                if j == 0:
                                    va = psv.tile([P, 512], F32, tag=f"v{i}",
                                                  bufs=1, name=f"va{ps}_{vg}_{i}")
                                    vaccs.append(va)
                                nc.tensor.matmul(
                                    vaccs[i][:],
                                    xt[kt][:, tt * P:(tt + 1) * P], wvt[kt][:],
                                    start=(j == 0), stop=(j == KPP - 1))
                        for i in range(4):
                            tt = vg * 4 + i
                            if ps == 0:
                                nc.scalar.copy(v_sb[tt][:], vaccs[i][:])
                            else:
                                nc.vector.tensor_tensor(
                                    v_sb[tt][:], v_sb[tt][:], vaccs[i][:],
                                    op=ADD)

                    # q,k: 8 (head, part) x 4 token-blocks
                    for h in range(HPC):
                        for part, wtens in (("q", wq), ("k", wk)):
                            if (part, h) not in w_tiles:
                                load_w(h, part, wtens)
                            wts = w_tiles[(part, h)]
                            for nb in range(NB):
                                dst = qk_sb[(part, h, nb)]
                                tsl = slice(nb * 512, (nb + 1) * 512)
                                acc = ps1.tile([P, 512], F32, tag="qk", bufs=4)
                                for j, kt in enumerate(kts):
                                    nc.tensor.matmul(
                                        acc[:], wts[j // (KPP // 2)][:, j % (KPP // 2), :],
                                        xt[kt][:, tsl],
                                        start=(j == 0),
                                        stop=(ps == 0 and j == KPP - 1))
                                if ps == 0:
                                    nc.scalar.copy(dst[:], acc[:])
                                else:
                                    nc.tensor.matmul(acc[:], eye_sb[:], dst[:],
                                                     start=False, stop=True)
                                    nc.scalar.copy(dst[:], acc[:])

            # ------------- Phase 2: attention fused with outproj ------------
            # jb outer / head inner; after each jb the output projection for
            # that token block runs, overlapping the next block's attention.
            with (
                tc.tile_pool(name="wop", bufs=1) as wopool,
                tc.tile_pool(name="ctxp", bufs=2) as ctxpool,
                tc.tile_pool(name="lp", bufs=2) as lpool,
                tc.tile_pool(name="pp", bufs=3) as ppool,
                tc.tile_pool(name="cxs", bufs=2) as cxspool,
                tc.tile_pool(name="yp", bufs=4) as ypool,
                tc.tile_pool(name="ps2s", bufs=4, space="PSUM") as ps2s,
                tc.tile_pool(name="ps2c", bufs=1, space="PSUM") as ps2c,
                tc.tile_pool(name="ps2l", bufs=1, space="PSUM") as ps2l,
                tc.tile_pool(name="ps3", bufs=2, space="PSUM") as ps3,
            ):
                # rope setup: rotate_half sign folded into sinT on host;
                # the half-swap is two SBUF->SBUF DMAs. Chains are emitted
                # inside the jb loop so per-engine streams interleave. Only
                # the nb=0 cos/sin chunks load before jb0's rope; wo and the
                # remaining chunks queue behind jb0's swaps.
                rope_cm1 = tc.tile_pool(name="rconst", bufs=1)
                rpool = rope_cm1.__enter__()
                rope_cm2 = tc.tile_pool(name="st", bufs=2)
                spool = rope_cm2.__enter__()
                cos_sb, sin_sb = [], []

                def load_cs(cnb):
                    csl = slice(cnb * 512, (cnb + 1) * 512)
                    c_ = rpool.tile([P, 512], F32, tag=f"cos{cnb}", name=f"cos{cnb}")
                    nc.sync.dma_start(c_[:], cosT[:, csl])
                    cos_sb.append(c_)
                    s_ = rpool.tile([P, 512], F32, tag=f"sin{cnb}", name=f"sin{cnb}")
                    nc.sync.dma_start(s_[:], sinT[:, csl])
                    sin_sb.append(s_)

                load_cs(0)
                half = P // 2
                wo_sb = []

                def rope_chunk(part, h, nb):
                    src = qk_sb[(part, h, nb)]
                    tmp = spool.tile([P, 512], dt, tag="rt", bufs=2, name="tmp")
                    nc.sync.dma_start(tmp[0:half, :], src[half:P, :])
                    nc.sync.dma_start(tmp[half:P, :], src[0:half, :])
                    t1 = spool.tile([P, 512], F32, tag="t1", bufs=2)
                    nc.gpsimd.tensor_tensor(t1[:], src[:], cos_sb[nb][:], op=MULT)
                    t2 = spool.tile([P, 512], F32, tag="t2", bufs=2)
                    nc.vector.tensor_tensor(t2[:], tmp[:], sin_sb[nb][:], op=MULT)
                    nc.vector.tensor_tensor(src[:], t1[:], t2[:], op=ADD)

                for jb in range(NB):
                    for h in range(HPC):
                        rope_chunk("k", h, jb)
                        rope_chunk("q", h, jb)
                    if jb == 0:
                        for cnb in range(1, NB):
                            load_cs(cnb)
                        for h in range(HPC):
                            w_sb = wopool.tile([P, C], dt, tag=f"wo{h}",
                                               name=f"wo{h}_sb")
                            nc.sync.dma_start(w_sb[:], wo[h])
                            wo_sb.append(w_sb)

                    nt = 4 * (jb + 1)
                    qsl = slice(jb * 512, (jb + 1) * 512)
                    ctx_tiles = {}
                    for h in range(HPC):
                        qT_sb = qk_sb[("q", h, jb)]
                        ctx_ps = ps2c.tile([P, 512], F32, tag="ctx", bufs=1)
                        lps = ps2l.tile([1, 512], F32, tag="l", bufs=1)
                        for i in range(nt):
                            r = i - 4 * jb
                            # causal narrowing: only tq >= tk contribute;
                            # r==3 keeps cols 256: with a memset for 256:384
                            c0 = 0 if r < 1 else (r * P if r <= 2 else 2 * P)
                            osl = slice(c0, 512)
                            kch = qk_sb[("k", h, i // 4)]
                            sps = ps2s.tile([P, 512], F32, tag="s", bufs=4)
                            nc.tensor.matmul(
                                sps[:, osl],
                                kch[:, (i % 4) * P:(i % 4 + 1) * P],
                                qT_sb[:, osl], start=True, stop=True)
                            pt = ppool.tile([P, 512], dt, tag="pt", bufs=6)
                            nc.scalar.activation(pt[:, osl], sps[:, osl], EXP,
                                                 scale=SCALE)
                            if 0 <= r <= 2:
                                dsl = slice(r * P, (r + 1) * P)
                                nc.gpsimd.tensor_tensor(
                                    pt[:, dsl], pt[:, dsl], tri_sb[:], op=MULT)
                            elif r == 3:
                                nc.gpsimd.tensor_tensor(
                                    pt[:, 2 * P:4 * P], pt[:, 2 * P:4 * P],
                                    m3_sb[:], op=MULT)
                            nc.tensor.matmul(
                                ctx_ps[:, osl],
                                v_sb[i][:, h * P:(h + 1) * P], pt[:, osl],
                                start=(i == 0), stop=(i == nt - 1))
                            nc.tensor.matmul(lps[:, osl], ones_sb[:, 0:1],
                                             pt[:, osl],
                                             start=(i == 0), stop=(i == nt - 1))
                        rinv = lpool.tile([1, 512], dt, tag="rinv", bufs=2)
                        with nc.allow_low_precision(reason="softmax 1/l fp32r"):
                            nc.vector.reciprocal(rinv[:], lps[:])
                        bps = ps3.tile([P, 512], F32, tag="y", bufs=2,
                                       name="bps")
                        nc.tensor.matmul(bps[:], ones_sb[0:1, :], rinv[:],
                                         start=True, stop=True)
                        cvt = cxspool.tile([P, 512], F32, tag="cvt")
                        nc.vector.tensor_copy(cvt[:], ctx_ps[:])
                        ctx_sb = ctxpool.tile([P, 512], dt, tag=f"cx{h}",
                                              bufs=2, name=f"ctx{h}_{jb}")
                        nc.vector.tensor_tensor(ctx_sb[:], cvt[:], bps[:], op=MULT)
                        ctx_tiles[h] = ctx_sb

                    # outproj for this token block
                    for sub in range(4):
                        tt = jb * 4 + sub
                        ssl = slice(sub * P, (sub + 1) * P)
                        for ob in range(NB):
                            yps = ps3.tile([P, 512], F32, tag="y", bufs=2)
                            for h in range(HPC):
                                nc.tensor.matmul(
                                    yps[:], ctx_tiles[h][:, ssl],
                                    wo_sb[h][:, ob * 512:(ob + 1) * 512],
                                    start=(h == 0), stop=(h == HPC - 1))
                            y_sb = ypool.tile([P, 512], F32, tag="ysb", bufs=4)
                            nc.vector.tensor_copy(y_sb[:], yps[:])
                            nc.sync.dma_start(
                                y[tt * P:(tt + 1) * P, ob * 512:(ob + 1) * 512],
                                y_sb[:])
                rope_cm2.__exit__(None, None, None)
                rope_cm1.__exit__(None, None, None)

    nc.compile()
    return nc


def _build_kernel(matmul_dt=F32R):
    key = str(matmul_dt)
    if key not in _CACHE:
        _CACHE[key] = _build(matmul_dt)
    return _CACHE[key]


def _host_constants():
    tri01 = (np.arange(P)[:, None] <= np.arange(P)[None, :]).astype(np.float32)
    m3 = np.concatenate([np.zeros((P, P), np.float32), tri01], axis=1)
    ones = np.ones((P, P), dtype=np.float32)
    eye = np.eye(P, dtype=np.float32)
    return tri01, m3, ones, eye


def prepare_in_maps(x, W_qkv, W_o, cos, sin):
    tri, m3, ones, eye = _host_constants()
    cosT = np.ascontiguousarray(cos.T)
    # rotate_half sign folded in: rows (head dims) 0..63 negated
    sgn = np.where(np.arange(P) < P // 2, -1.0, 1.0).astype(np.float32)
    sinT = np.ascontiguousarray(sin.T * sgn[:, None])

    in_maps = []
    for core in range(NCORES):
        b = core // 4
        hg0 = (core % 4) * HPC
        rows = slice(hg0 * P, (hg0 + HPC) * P)
        xTc = np.ascontiguousarray(x[b].T)
        wq_r = W_qkv[0 * C:1 * C][rows]        # [512, 2048]
        wk_r = W_qkv[1 * C:2 * C][rows]
        wv_r = W_qkv[2 * C:3 * C][rows]
        # (h, c_in_tile, kt, d) from W^T [2048(c), 512(h,d)]
        wq_t = np.ascontiguousarray(
            wq_r.T.reshape(NKT, P, HPC, P).transpose(2, 1, 0, 3))
        wk_t = np.ascontiguousarray(
            wk_r.T.reshape(NKT, P, HPC, P).transpose(2, 1, 0, 3))
        wv_t = np.ascontiguousarray(wv_r.T.reshape(NKT, P, HPC * P))
        wo_t = np.ascontiguousarray(W_o[:, rows].T.reshape(HPC, P, C))
        in_maps.append({
            "xT": xTc, "wq": wq_t, "wk": wk_t, "wv": wv_t, "wo": wo_t,
            "cosT": cosT, "sinT": sinT, "tri": tri, "m3": m3, "ones": ones, "eye": eye,
        })
    return in_maps


def gather(results, b_o):
    y = np.zeros((2, T, C), dtype=np.float32)
    for core in range(NCORES):
        y[core // 4] += results[core]["y"]
    y += np.asarray(b_o, dtype=np.float32)[None, None, :]
    return y


def kernel(x, W_qkv, W_o, b_o, cos, sin):
    x = np.asarray(x, dtype=np.float32)
    W_qkv = np.asarray(W_qkv, dtype=np.float32)
    W_o = np.asarray(W_o, dtype=np.float32)
    cos = np.asarray(cos, dtype=np.float32)
    sin = np.asarray(sin, dtype=np.float32)
    nc = _build_kernel()
    in_maps = prepare_in_maps(x, W_qkv, W_o, cos, sin)
    res = run_bass_kernel_spmd(nc, in_maps, core_ids=list(range(NCORES)))
    return gather(res.results, b_o)



# revision 2
# speedup vs baseline: 1.2172x; 1.2172x over previous
"""Causal multi-head attention (RoPE) on 8 TRN2 NeuronCores.

Problem: x[2,2048,2048] -> qkv proj -> rope -> causal attention (16 heads,
head_dim 128) -> output proj + bias. Sharding: (batch, head-group) across the
8 cores - core c handles batch c//4 and heads 4*(c%4)..4*(c%4)+3. Each core
computes a partial output projection over its heads' channels; the host sums
the 4 partials per batch and adds b_o.

Single-pass token-outer pipeline, everything fp16 on device (PSUM accumulation
stays f32; final host reduction in f32; validated rel err ~5e-4 vs the fp32
reference). Per 512-token block nb:
  - QKV projection with kt (contraction tile) outer so each x tile is consumed
    on arrival: q,k for 4 heads accumulate in 8 PSUM banks, evict (ACT) to
    SBUF fp16, rope applied in place (half-swap via 2 small SBUF DMAs, sign
    folded into sinT host-side; mults/adds split Pool/DVE); then v for 4
    token sub-tiles reuses 4 banks while rope drains on the side engines.
  - Attention for query block jb=nb over key tiles 0..4(nb+1)-1, scores
    transposed s^T[tk,tq] (lhsT=k tile, rhs=q block) with causal narrowing;
    exp on ACT -> pt fp16; diagonal tri-mask on Pool; AV accumulates
    ctx^T[d,tq] in PSUM. Softmax denominators: pt tiles are element-wise
    accumulated on DVE (fp16, 2x mode) into lacc[128,tq]; ONE ones-matmul per
    (head, block) broadcasts the partition sum to l[128,tq] in PSUM (the old
    per-tile ones-matmuls burned ~30us of PE). DVE reciprocal + one multiply
    normalize ctx straight out of PSUM.
  - Output projection per 128-token sub-tile accumulates the 4 heads in PSUM,
    evicts quarters to an fp16 [128,2048] staging tile, one DMA per sub-tile.

Matmul cost on this target is (moving columns) x cycles/row with the dtype
taken from the MOVING operand; fp16 runs 1 cycle/row with no 256-column
minimum (f32r needs >=256), so the r=3 diagonal score tile narrows to 128
columns and the m3 mask is gone. DMAs are batched (multi-kt x/weight loads,
row-block y stores) to cut HWDGE/queue overhead.
"""
import math

import numpy as np

import concourse.bacc as bacc
import concourse.mybir as mybir
import concourse.tile as tile
from concourse.bass_utils import run_bass_kernel_spmd

P = 128           # partitions / head_dim
T = 2048          # context length
C = 2048          # d_model
NKT = C // P      # 16 contraction tiles
NB = T // 512     # 4 token blocks of 512
HPC = 4           # heads per core
NCORES = 8
SCALE = 1.0 / math.sqrt(P)

F32 = mybir.dt.float32
F16 = mybir.dt.float16
EXP = mybir.ActivationFunctionType.Exp
MULT = mybir.AluOpType.mult
ADD = mybir.AluOpType.add

_CACHE = {}


def _build():
    nc = bacc.Bacc("TRN2", target_bir_lowering=False, debug=False,
                   num_devices=NCORES)
    xg = nc.dram_tensor("xg", (P, NKT, T), F16, kind="ExternalInput").ap()
    wq = nc.dram_tensor("wq", (P, NKT, HPC * P), F16, kind="ExternalInput").ap()
    wk = nc.dram_tensor("wk", (P, NKT, HPC * P), F16, kind="ExternalInput").ap()
    wv = nc.dram_tensor("wv", (P, NKT, HPC * P), F16, kind="ExternalInput").ap()
    wo = nc.dram_tensor("wo", (P, HPC, C), F16, kind="ExternalInput").ap()
    cosT = nc.dram_tensor("cosT", (P, T), F16, kind="ExternalInput").ap()
    sinT = nc.dram_tensor("sinT", (P, T), F16, kind="ExternalInput").ap()
    tri = nc.dram_tensor("tri", (P, P), F16, kind="ExternalInput").ap()
    ones = nc.dram_tensor("ones", (P, P), F16, kind="ExternalInput").ap()
    y = nc.dram_tensor("y", (T, C), F16, kind="ExternalOutput").ap()

    half = P // 2

    with tile.TileContext(nc) as tc:
        with (
            tc.tile_pool(name="gconst", bufs=1) as gpool,
            tc.tile_pool(name="wbuf", bufs=1) as wpool,
            tc.tile_pool(name="xbuf", bufs=1) as xpool,
            tc.tile_pool(name="qkbuf", bufs=1) as qkpool,
            tc.tile_pool(name="vbuf", bufs=1) as vpool,
            tc.tile_pool(name="rope", bufs=1) as rpool,
            tc.tile_pool(name="ptb", bufs=1) as ptpool,
            tc.tile_pool(name="stats", bufs=1) as spool,
            tc.tile_pool(name="ctxb", bufs=1) as cxpool,
            tc.tile_pool(name="yb", bufs=1) as ypool,
            tc.tile_pool(name="ps", bufs=1, space="PSUM") as ps,
        ):
            tri_sb = gpool.tile([P, P], F16, tag="tri")
            ones_sb = gpool.tile([P, P], F16, tag="ones")
            wq_sb = wpool.tile([P, NKT, HPC * P], F16, tag="wq", name="wq_sb")
            wk_sb = wpool.tile([P, NKT, HPC * P], F16, tag="wk", name="wk_sb")
            wv_sb = wpool.tile([P, NKT, HPC * P], F16, tag="wv", name="wv_sb")
            wo_sb = wpool.tile([P, HPC, C], F16, tag="wo", name="wo_sb")
            cos_sb = wpool.tile([P, T], F16, tag="cos", name="cos_sb")
            sin_sb = wpool.tile([P, T], F16, tag="sin", name="sin_sb")

            qk_sb = {}
            for h in range(HPC):
                for part in ("q", "k"):
                    for nb in range(NB):
                        qk_sb[(part, h, nb)] = qkpool.tile(
                            [P, 512], F16, tag=f"{part}{h}n{nb}",
                            name=f"{part}{h}n{nb}_sb")
            v_sb = [vpool.tile([P, 512], F16, tag=f"vb{i}", name=f"v{i}_sb")
                    for i in range(NKT)]

            # startup DMAs: x(nb=0) + wq + wk interleaved in 4-kt quartets so
            # the kt-outer projection starts ~4us in; the rest queues behind.
            x0 = xpool.tile([P, NKT, 512], F16, tag="x", bufs=2, name="x_nb0")
            for g in range(0, NKT, 4):
                gs = slice(g, g + 4)
                nc.sync.dma_start(x0[:, gs, :], xg[:, gs, 0:512])
                nc.sync.dma_start(wq_sb[:, gs, :], wq[:, gs, :])
                nc.sync.dma_start(wk_sb[:, gs, :], wk[:, gs, :])
            nc.sync.dma_start(cos_sb[:], cosT)
            nc.sync.dma_start(sin_sb[:], sinT)
            nc.sync.dma_start(wv_sb[:], wv)
            nc.sync.dma_start(tri_sb[:], tri)
            nc.sync.dma_start(ones_sb[:], ones)
            nc.sync.dma_start(wo_sb[:], wo)

            xt = x0
            for nb in range(NB):
                nsl = slice(nb * 512, (nb + 1) * 512)
                # ---- q,k projection for this token block (kt outer) ----
                accs = {}
                for j, (part, h) in enumerate(
                        (p_, h_) for h_ in range(HPC) for p_ in ("q", "k")):
                    accs[(part, h)] = ps.tile([P, 512], F32, tag=f"b{j}",
                                              name=f"qk{part}{h}_{nb}")
                for kt in range(NKT):
                    for part, wsb in (("q", wq_sb), ("k", wk_sb)):
                        for h in range(HPC):
                            nc.tensor.matmul(
                                accs[(part, h)][:],
                                wsb[:, kt, h * P:(h + 1) * P],
                                xt[:, kt, :],
                                start=(kt == 0), stop=(kt == NKT - 1))
                # evict + rope (in place); swap halves via 2 small SBUF DMAs
                for h in range(HPC):
                    for part in ("k", "q"):
                        dst = qk_sb[(part, h, nb)]
                        nc.scalar.copy(dst[:], accs[(part, h)][:])
                        tmp = rpool.tile([P, 512], F16, tag="rt", bufs=2,
                                         name="rtmp")
                        nc.sync.dma_start(tmp[0:half, :], dst[half:P, :])
                        nc.sync.dma_start(tmp[half:P, :], dst[0:half, :])
                        t1 = rpool.tile([P, 512], F16, tag="t1", bufs=2)
                        nc.gpsimd.tensor_tensor(t1[:], dst[:], cos_sb[:, nsl],
                                                op=MULT)
                        t2 = rpool.tile([P, 512], F16, tag="t2", bufs=2)
                        nc.vector.tensor_tensor(t2[:], tmp[:], sin_sb[:, nsl],
                                                op=MULT)
                        nc.vector.tensor_tensor(dst[:], t1[:], t2[:], op=ADD)

                # ---- v projection (reuses 4 PSUM banks) ----
                vaccs = [ps.tile([P, 512], F32, tag=f"b{i}", name=f"va{nb}_{i}")
                         for i in range(4)]
                for kt in range(NKT):
                    for tt in range(4):
                        nc.tensor.matmul(
                            vaccs[tt][:],
                            xt[:, kt, tt * P:(tt + 1) * P],
                            wv_sb[:, kt, :],
                            start=(kt == 0), stop=(kt == NKT - 1))
                for tt in range(4):
                    nc.scalar.copy(v_sb[nb * 4 + tt][:], vaccs[tt][:])

                # prefetch next block's x tiles (downloads during attention)
                if nb < NB - 1:
                    xt = xpool.tile([P, NKT, 512], F16, tag="x", bufs=2,
                                    name=f"x_nb{nb + 1}")
                    nc.sync.dma_start(
                        xt[:], xg[:, :, (nb + 1) * 512:(nb + 2) * 512])

                # ---- attention for query block jb = nb ----
                nt = 4 * (nb + 1)
                ctx_tiles = {}
                for h in range(HPC):
                    qT = qk_sb[("q", h, nb)]
                    ctx_ps = ps.tile([P, 512], F32, tag="b7", name=f"ctx{h}_{nb}")
                    lacc = spool.tile([P, 512], F16, tag="lacc", bufs=2)
                    for i in range(nt):
                        r = i - 4 * nb
                        c0 = 0 if r < 1 else r * P
                        osl = slice(c0, 512)
                        kch = qk_sb[("k", h, i // 4)]
                        sps = ps.tile([P, 512], F32, tag=f"b{4 + i % 3}",
                                      name=f"s{h}_{nb}_{i}")
                        nc.tensor.matmul(
                            sps[:, osl],
                            kch[:, (i % 4) * P:(i % 4 + 1) * P],
                            qT[:, osl], start=True, stop=True)
                        pt = ptpool.tile([P, 512], F16, tag="pt", bufs=6)
                        nc.scalar.activation(pt[:, osl], sps[:, osl], EXP,
                                             scale=SCALE)
                        if r >= 0:
                            dsl = slice(r * P, (r + 1) * P)
                            nc.gpsimd.tensor_tensor(
                                pt[:, dsl], pt[:, dsl], tri_sb[:], op=MULT)
                        nc.tensor.matmul(
                            ctx_ps[:, osl],
                            v_sb[i][:, h * P:(h + 1) * P], pt[:, osl],
                            start=(i == 0), stop=(i == nt - 1))
                        if i == 0:
                            nc.vector.tensor_copy(lacc[:], pt[:])
                        else:
                            nc.vector.tensor_tensor(
                                lacc[:, osl], lacc[:, osl], pt[:, osl], op=ADD)
                    # one partition-sum+broadcast matmul per (head, block)
                    lb = ps.tile([P, 512], F32, tag="b0", name=f"l{h}_{nb}")
                    nc.tensor.matmul(lb[:], ones_sb[:], lacc[:],
                                     start=True, stop=True)
                    rinv = spool.tile([P, 512], F32, tag="rinv", bufs=2)
                    nc.vector.reciprocal(rinv[:], lb[:])
                    ctx_sb = cxpool.tile([P, 512], F16, tag=f"cx{h}", bufs=2,
                                         name=f"cs{h}_{nb}")
                    nc.vector.tensor_tensor(ctx_sb[:], ctx_ps[:], rinv[:],
                                            op=MULT)
                    ctx_tiles[h] = ctx_sb

                # ---- output projection for this token block ----
                for sub in range(4):
                    tt = nb * 4 + sub
                    ssl = slice(sub * P, (sub + 1) * P)
                    ysb = ypool.tile([P, C], F16, tag="ysb", bufs=2)
                    for ob in range(NB):
                        obsl = slice(ob * 512, (ob + 1) * 512)
                        yps = ps.tile([P, 512], F32, tag=f"b{1 + ob % 2}",
                                      name=f"y{tt}_{ob}")
                        for h in range(HPC):
                            nc.tensor.matmul(
                                yps[:], ctx_tiles[h][:, ssl],
                                wo_sb[:, h, obsl],
                                start=(h == 0), stop=(h == HPC - 1))
                        nc.vector.tensor_copy(ysb[:, obsl], yps[:])
                    nc.sync.dma_start(y[tt * P:(tt + 1) * P, :], ysb[:])

    nc.compile()
    return nc


def _build_kernel():
    if "k" not in _CACHE:
        _CACHE["k"] = _build()
    return _CACHE["k"]


def prepare_in_maps(x, W_qkv, W_o, cos, sin):
    f16 = np.float16
    tri01 = (np.arange(P)[:, None] <= np.arange(P)[None, :]).astype(f16)
    ones = np.ones((P, P), dtype=f16)
    cosT = np.ascontiguousarray(cos.T).astype(f16)
    # rotate_half sign folded in: rows (head dims) 0..63 negated
    sgn = np.where(np.arange(P) < P // 2, -1.0, 1.0).astype(np.float32)
    sinT = (cos.T * 0 + sin.T * sgn[:, None]).astype(f16)

    in_maps = []
    for core in range(NCORES):
        b = core // 4
        hg0 = (core % 4) * HPC
        rows = slice(hg0 * P, (hg0 + HPC) * P)
        xT = x[b].T  # [C, T]
        xg = np.ascontiguousarray(
            xT.reshape(NKT, P, T).transpose(1, 0, 2)).astype(f16)

        def wprep(w):  # [512 rows, 2048 c] -> [P, NKT, 512]
            return np.ascontiguousarray(
                w.T.reshape(NKT, P, HPC * P).transpose(1, 0, 2)).astype(f16)

        wq_t = wprep(W_qkv[0 * C:1 * C][rows])
        wk_t = wprep(W_qkv[1 * C:2 * C][rows])
        wv_t = wprep(W_qkv[2 * C:3 * C][rows])
        wo_t = np.ascontiguousarray(
            W_o[:, rows].T.reshape(HPC, P, C).transpose(1, 0, 2)).astype(f16)
        in_maps.append({
            "xg": xg, "wq": wq_t, "wk": wk_t, "wv": wv_t, "wo": wo_t,
            "cosT": cosT, "sinT": sinT, "tri": tri01, "ones": ones,
        })
    return in_maps


def gather(results, b_o):
    y = np.zeros((2, T, C), dtype=np.float32)
    for core in range(NCORES):
        y[core // 4] += results[core]["y"].astype(np.float32)
    y += np.asarray(b_o, dtype=np.float32)[None, None, :]
    return y


def kernel(x, W_qkv, W_o, b_o, cos, sin):
    x = np.asarray(x, dtype=np.float32)
    W_qkv = np.asarray(W_qkv, dtype=np.float32)
    W_o = np.asarray(W_o, dtype=np.float32)
    cos = np.asarray(cos, dtype=np.float32)
    sin = np.asarray(sin, dtype=np.float32)
    nc = _build_kernel()
    in_maps = prepare_in_maps(x, W_qkv, W_o, cos, sin)
    res = run_bass_kernel_spmd(nc, in_maps, core_ids=list(range(NCORES)))
    return gather(res.results, b_o)


# revision 3
# speedup vs baseline: 1.2251x; 1.0065x over previous
"""Causal multi-head attention (RoPE) on 8 TRN2 NeuronCores.

Problem: x[2,2048,2048] -> qkv proj -> rope -> causal attention (16 heads,
head_dim 128) -> output proj + bias. Sharding: (batch, head-group) across the
8 cores - core c handles batch c//4 and heads 4*(c%4)..4*(c%4)+3. Each core
computes a partial output projection over its heads' channels; the host sums
the 4 partials per batch and adds b_o.

Single-pass token-outer pipeline, everything fp16 on device (PSUM accumulation
stays f32; final host reduction in f32; validated rel err ~5e-4 vs the fp32
reference). The exp throughput on ACT (0.833ns/col) exactly matches the
scores+AV cost on PE, so attention phases are ACT-bound unless PE borrows
other work: the next block's QKV projection is software-pipelined INTO the
attention window as three waves:

    attn(nb) heads -> q-wave(nb+1) -> outproj(nb) -> k-wave(nb+1)
                   -> v-wave(nb+1) -> attn(nb+1) ...

Waves are accumulator-major (16 kt matmuls per PSUM bank) with a bank map
chosen so each wave's first banks were freed earliest by the previous phase:
q-wave on b1,b2,b0,b7 / k-wave on b3..b6 / v-wave on b0,b7,b1,b2; attention
rotates scores over b3..b6 4-deep (tag b{3+(i+h)%4}), softmax-denominator
broadcast lb takes the next slot in that rotation, ctx alternates b0/b7.

Scores are transposed s^T[tk,tq] (lhsT=k tile, rhs=q block) with causal
narrowing; matmul cost here is (moving columns) x (cycles/row keyed on the
MOVING operand dtype): fp16 runs 1 cycle/row with no 256-column minimum, so
the r=3 diagonal tile narrows to 128 columns. Softmax denominators come from
element-wise fp16 accumulation of the exp tiles on DVE (2x mode) + ONE
ones-matmul per (head, block) that broadcasts the partition sum - the
per-tile [1,512] ones-matmuls this replaces cost a full 30us of PE. RoPE is
applied in place (half-swap via 2 small SBUF DMAs, sign folded into sinT on
the host; mults split Pool/DVE). Output projection accumulates the 4 heads
in PSUM per 128-token sub-tile; quarter evictions alternate DVE/ACT into an
fp16 [128,2048] staging row, one DMA per sub-tile (per-quarter DMAs on the
last block to shorten the tail). DMAs are batched multi-kt loads; x for block
nb+1 prefetches during attention nb.
"""
import math

import numpy as np

import concourse.bacc as bacc
import concourse.mybir as mybir
import concourse.tile as tile
from concourse.bass_utils import run_bass_kernel_spmd

P = 128           # partitions / head_dim
T = 2048          # context length
C = 2048          # d_model
NKT = C // P      # 16 contraction tiles
NB = T // 512     # 4 token blocks of 512
HPC = 4           # heads per core
NCORES = 8
SCALE = 1.0 / math.sqrt(P)

F32 = mybir.dt.float32
F16 = mybir.dt.float16
EXP = mybir.ActivationFunctionType.Exp
MULT = mybir.AluOpType.mult
ADD = mybir.AluOpType.add

QBANKS = ("b1", "b2", "b0", "b7")   # q-wave accumulators, emission order
KBANKS = ("b3", "b4", "b5", "b6")   # k-wave accumulators
VBANKS = ("b0", "b7", "b1", "b2")   # v-wave accumulators
CTXBANKS = ("b0", "b7")             # ctx_ps alternates by head parity

_CACHE = {}


def _build():
    nc = bacc.Bacc("TRN2", target_bir_lowering=False, debug=False,
                   num_devices=NCORES)
    xg = nc.dram_tensor("xg", (P, NKT, T), F16, kind="ExternalInput").ap()
    wq = nc.dram_tensor("wq", (P, NKT, HPC * P), F16, kind="ExternalInput").ap()
    wk = nc.dram_tensor("wk", (P, NKT, HPC * P), F16, kind="ExternalInput").ap()
    wv = nc.dram_tensor("wv", (P, NKT, HPC * P), F16, kind="ExternalInput").ap()
    wo = nc.dram_tensor("wo", (P, HPC, C), F16, kind="ExternalInput").ap()
    cosT = nc.dram_tensor("cosT", (P, T), F16, kind="ExternalInput").ap()
    sinT = nc.dram_tensor("sinT", (P, T), F16, kind="ExternalInput").ap()
    tri = nc.dram_tensor("tri", (P, P), F16, kind="ExternalInput").ap()
    ones = nc.dram_tensor("ones", (P, P), F16, kind="ExternalInput").ap()
    y = nc.dram_tensor("y", (T, C), F16, kind="ExternalOutput").ap()

    half = P // 2

    with tile.TileContext(nc) as tc:
        with (
            tc.tile_pool(name="gconst", bufs=1) as gpool,
            tc.tile_pool(name="wbuf", bufs=1) as wpool,
            tc.tile_pool(name="xbuf", bufs=1) as xpool,
            tc.tile_pool(name="qkbuf", bufs=1) as qkpool,
            tc.tile_pool(name="vbuf", bufs=1) as vpool,
            tc.tile_pool(name="rope", bufs=1) as rpool,
            tc.tile_pool(name="ptb", bufs=1) as ptpool,
            tc.tile_pool(name="stats", bufs=1) as spool,
            tc.tile_pool(name="ctxb", bufs=1) as cxpool,
            tc.tile_pool(name="yb", bufs=1) as ypool,
            tc.tile_pool(name="ps", bufs=1, space="PSUM") as ps,
        ):
            tri_sb = gpool.tile([P, P], F16, tag="tri")
            ones_sb = gpool.tile([P, P], F16, tag="ones")
            wq_sb = wpool.tile([P, NKT, HPC * P], F16, tag="wq", name="wq_sb")
            wk_sb = wpool.tile([P, NKT, HPC * P], F16, tag="wk", name="wk_sb")
            wv_sb = wpool.tile([P, NKT, HPC * P], F16, tag="wv", name="wv_sb")
            wo_sb = wpool.tile([P, HPC, C], F16, tag="wo", name="wo_sb")
            cos_sb = wpool.tile([P, T], F16, tag="cos", name="cos_sb")
            sin_sb = wpool.tile([P, T], F16, tag="sin", name="sin_sb")

            qk_sb = {}
            for h in range(HPC):
                for part in ("q", "k"):
                    for nb in range(NB):
                        qk_sb[(part, h, nb)] = qkpool.tile(
                            [P, 512], F16, tag=f"{part}{h}n{nb}",
                            name=f"{part}{h}n{nb}_sb")
            v_sb = [vpool.tile([P, 512], F16, tag=f"vb{i}", name=f"v{i}_sb")
                    for i in range(NKT)]

            def wave_accs(banks, label):
                return [ps.tile([P, 512], F32, tag=banks[h],
                                name=f"{label}{h}") for h in range(HPC)]

            def evict_rope(nb, part, accs):
                nsl = slice(nb * 512, (nb + 1) * 512)
                for h in range(HPC):
                    dst = qk_sb[(part, h, nb)]
                    nc.scalar.copy(dst[:], accs[h][:])
                    tmp = rpool.tile([P, 512], F16, tag="rt", bufs=2,
                                     name="rtmp")
                    nc.sync.dma_start(tmp[0:half, :], dst[half:P, :])
                    nc.sync.dma_start(tmp[half:P, :], dst[0:half, :])
                    t1 = rpool.tile([P, 512], F16, tag="t1", bufs=2)
                    nc.gpsimd.tensor_tensor(t1[:], dst[:], cos_sb[:, nsl],
                                            op=MULT)
                    t2 = rpool.tile([P, 512], F16, tag="t2", bufs=2)
                    nc.vector.tensor_tensor(t2[:], tmp[:], sin_sb[:, nsl],
                                            op=MULT)
                    nc.vector.tensor_tensor(dst[:], t1[:], t2[:], op=ADD)

            def qk_wave(part, xt):
                wsb = wq_sb if part == "q" else wk_sb
                accs = wave_accs(QBANKS if part == "q" else KBANKS, part)
                for h in range(HPC):
                    for kt in range(NKT):
                        nc.tensor.matmul(
                            accs[h][:], wsb[:, kt, h * P:(h + 1) * P],
                            xt[:, kt, :], start=(kt == 0), stop=(kt == NKT - 1))
                return accs

            def v_wave(nb, xt):
                vaccs = wave_accs(VBANKS, f"v{nb}_")
                for tt in range(4):
                    for kt in range(NKT):
                        nc.tensor.matmul(
                            vaccs[tt][:], xt[:, kt, tt * P:(tt + 1) * P],
                            wv_sb[:, kt, :], start=(kt == 0),
                            stop=(kt == NKT - 1))
                for tt in range(4):
                    nc.scalar.copy(v_sb[nb * 4 + tt][:], vaccs[tt][:])

            def attention(nb):
                nt = 4 * (nb + 1)
                ctx_tiles = {}
                for h in range(HPC):
                    qT = qk_sb[("q", h, nb)]
                    ctx_ps = ps.tile([P, 512], F32, tag=CTXBANKS[h % 2],
                                     name=f"ctx{h}_{nb}")
                    lacc = spool.tile([P, 512], F16, tag="lacc", bufs=2)
                    for i in range(nt):
                        r = i - 4 * nb
                        c0 = 0 if r < 1 else r * P
                        osl = slice(c0, 512)
                        kch = qk_sb[("k", h, i // 4)]
                        sps = ps.tile([P, 512], F32, tag=f"b{3 + (i + h) % 4}",
                                      name=f"s{h}_{nb}_{i}")
                        nc.tensor.matmul(
                            sps[:, osl],
                            kch[:, (i % 4) * P:(i % 4 + 1) * P],
                            qT[:, osl], start=True, stop=True)
                        pt = ptpool.tile([P, 512], F16, tag="pt", bufs=6)
                        nc.scalar.activation(pt[:, osl], sps[:, osl], EXP,
                                             scale=SCALE)
                        if r >= 0:
                            dsl = slice(r * P, (r + 1) * P)
                            nc.gpsimd.tensor_tensor(
                                pt[:, dsl], pt[:, dsl], tri_sb[:], op=MULT)
                        nc.tensor.matmul(
                            ctx_ps[:, osl],
                            v_sb[i][:, h * P:(h + 1) * P], pt[:, osl],
                            start=(i == 0), stop=(i == nt - 1))
                        if i == 0:
                            nc.vector.tensor_copy(lacc[:], pt[:])
                        else:
                            nc.vector.tensor_tensor(
                                lacc[:, osl], lacc[:, osl], pt[:, osl], op=ADD)
                    # one partition-sum+broadcast matmul per (head, block)
                    lb = ps.tile([P, 512], F32, tag=f"b{3 + h % 4}",
                                 name=f"l{h}_{nb}")
                    nc.tensor.matmul(lb[:], ones_sb[:], lacc[:],
                                     start=True, stop=True)
                    rinv = spool.tile([P, 512], F32, tag="rinv", bufs=2)
                    nc.vector.reciprocal(rinv[:], lb[:])
                    ctx_sb = cxpool.tile([P, 512], F16, tag=f"cx{h}", bufs=2,
                                         name=f"cs{h}_{nb}")
                    nc.vector.tensor_tensor(ctx_sb[:], ctx_ps[:], rinv[:],
                                            op=MULT)
                    ctx_tiles[h] = ctx_sb
                return ctx_tiles

            def outproj(nb, ctx_tiles):
                last = nb == NB - 1
                for sub in range(4):
                    tt = nb * 4 + sub
                    ssl = slice(sub * P, (sub + 1) * P)
                    ysb = ypool.tile([P, C], F16, tag="ysb", bufs=2)
                    for ob in range(NB):
                        obsl = slice(ob * 512, (ob + 1) * 512)
                        yps = ps.tile([P, 512], F32, tag=f"b{1 + ob % 2}",
                                      name=f"y{tt}_{ob}")
                        for h in range(HPC):
                            nc.tensor.matmul(
                                yps[:], ctx_tiles[h][:, ssl],
                                wo_sb[:, h, obsl],
                                start=(h == 0), stop=(h == HPC - 1))
                        if ob % 2 == 0:
                            nc.vector.tensor_copy(ysb[:, obsl], yps[:])
                        else:
                            nc.scalar.copy(ysb[:, obsl], yps[:])
                        if last:
                            nc.sync.dma_start(
                                y[tt * P:(tt + 1) * P, obsl], ysb[:, obsl])
                    if not last:
                        nc.sync.dma_start(y[tt * P:(tt + 1) * P, :], ysb[:])

            # ---- startup DMAs: x(0)+wq+wk interleaved, fine-grained head ----
            x0 = xpool.tile([P, NKT, 512], F16, tag="x", bufs=2, name="x_nb0")
            groups = [(0, 2), (2, 4), (4, 8), (8, 12), (12, 16)]
            for lo, hi in groups:
                gs = slice(lo, hi)
                nc.sync.dma_start(x0[:, gs, :], xg[:, gs, 0:512])
                nc.sync.dma_start(wq_sb[:, gs, :], wq[:, gs, :])
                nc.sync.dma_start(wk_sb[:, gs, :], wk[:, gs, :])
            nc.sync.dma_start(cos_sb[:], cosT)
            nc.sync.dma_start(sin_sb[:], sinT)
            nc.sync.dma_start(wv_sb[:], wv)
            nc.sync.dma_start(tri_sb[:], tri)
            nc.sync.dma_start(ones_sb[:], ones)
            nc.sync.dma_start(wo_sb[:], wo)

            # ---- block 0: plain qk (kt-outer tracks DMA arrival), v ----
            qaccs = wave_accs(QBANKS, "q")
            kaccs = wave_accs(KBANKS, "k")
            for kt in range(NKT):
                for part, wsb, accs in (("q", wq_sb, qaccs),
                                        ("k", wk_sb, kaccs)):
                    for h in range(HPC):
                        nc.tensor.matmul(
                            accs[h][:], wsb[:, kt, h * P:(h + 1) * P],
                            x0[:, kt, :], start=(kt == 0), stop=(kt == NKT - 1))
            evict_rope(0, "k", kaccs)
            evict_rope(0, "q", qaccs)
            v_wave(0, x0)

            xt = x0
            for nb in range(NB):
                # prefetch next block's x during attention
                if nb < NB - 1:
                    xt = xpool.tile([P, NKT, 512], F16, tag="x", bufs=2,
                                    name=f"x_nb{nb + 1}")
                    nc.sync.dma_start(
                        xt[:], xg[:, :, (nb + 1) * 512:(nb + 2) * 512])
                ctx_tiles = attention(nb)
                if nb < NB - 1:
                    qaccs = qk_wave("q", xt)
                    evict_rope(nb + 1, "q", qaccs)
                outproj(nb, ctx_tiles)
                if nb < NB - 1:
                    kaccs = qk_wave("k", xt)
                    evict_rope(nb + 1, "k", kaccs)
                    v_wave(nb + 1, xt)

    nc.compile()
    return nc


def _build_kernel():
    if "k" not in _CACHE:
        _CACHE["k"] = _build()
    return _CACHE["k"]


def prepare_in_maps(x, W_qkv, W_o, cos, sin):
    f16 = np.float16
    tri01 = (np.arange(P)[:, None] <= np.arange(P)[None, :]).astype(f16)
    ones = np.ones((P, P), dtype=f16)
    cosT = np.ascontiguousarray(cos.T).astype(f16)
    # rotate_half sign folded in: rows (head dims) 0..63 negated
    sgn = np.where(np.arange(P) < P // 2, -1.0, 1.0).astype(np.float32)
    sinT = (sin.T * sgn[:, None]).astype(f16)

    in_maps = []
    for core in range(NCORES):
        b = core // 4
        hg0 = (core % 4) * HPC
        rows = slice(hg0 * P, (hg0 + HPC) * P)
        xT = x[b].T  # [C, T]
        xg = np.ascontiguousarray(
            xT.reshape(NKT, P, T).transpose(1, 0, 2)).astype(f16)

        def wprep(w):  # [512 rows, 2048 c] -> [P, NKT, 512]
            return np.ascontiguousarray(
                w.T.reshape(NKT, P, HPC * P).transpose(1, 0, 2)).astype(f16)

        wq_t = wprep(W_qkv[0 * C:1 * C][rows])
        wk_t = wprep(W_qkv[1 * C:2 * C][rows])
        wv_t = wprep(W_qkv[2 * C:3 * C][rows])
        wo_t = np.ascontiguousarray(
            W_o[:, rows].T.reshape(HPC, P, C).transpose(1, 0, 2)).astype(f16)
        in_maps.append({
            "xg": xg, "wq": wq_t, "wk": wk_t, "wv": wv_t, "wo": wo_t,
            "cosT": cosT, "sinT": sinT, "tri": tri01, "ones": ones,
        })
    return in_maps


def gather(results, b_o):
    y = np.zeros((2, T, C), dtype=np.float32)
    for core in range(NCORES):
        y[core // 4] += results[core]["y"].astype(np.float32)
    y += np.asarray(b_o, dtype=np.float32)[None, None, :]
    return y


def kernel(x, W_qkv, W_o, b_o, cos, sin):
    x = np.asarray(x, dtype=np.float32)
    W_qkv = np.asarray(W_qkv, dtype=np.float32)
    W_o = np.asarray(W_o, dtype=np.float32)
    cos = np.asarray(cos, dtype=np.float32)
    sin = np.asarray(sin, dtype=np.float32)
    nc = _build_kernel()
    in_maps = prepare_in_maps(x, W_qkv, W_o, cos, sin)
    res = run_bass_kernel_spmd(nc, in_maps, core_ids=list(range(NCORES)))
    return gather(res.results, b_o)


# revision 10
# speedup vs baseline: 1.2635x; 1.0313x over previous
"""Causal multi-head attention (RoPE) on 8 TRN2 NeuronCores.

Problem: x[2,2048,2048] -> qkv proj -> rope -> causal attention (16 heads,
head_dim 128) -> output proj + bias. Sharding: (batch, head-group) across the
8 cores - core c handles batch c//4 and heads 4*(c%4)..4*(c%4)+3. Each core
computes a partial output projection over its heads' channels; the host sums
the 4 partials per batch and adds b_o.

Single-pass token-outer pipeline, everything fp16 on device (PSUM accumulation
stays f32; final host reduction in f32; validated rel err ~5e-4 vs the fp32
reference). The exp throughput on ACT (0.833ns/col) exactly matches the
scores+AV cost on PE, so attention phases are ACT-bound unless PE borrows
other work: the next block's QKV projection is software-pipelined INTO the
attention window as three waves:

    attn(nb) heads -> q-wave(nb+1) -> outproj(nb) -> k-wave(nb+1)
                   -> v-wave(nb+1) -> attn(nb+1) ...

Waves are accumulator-major (16 kt matmuls per PSUM bank) with a bank map
chosen so each wave's first banks were freed earliest by the previous phase:
q-wave on b1,b2,b0,b7 / k-wave on b3..b6 / v-wave on b0,b7,b1,b2; attention
rotates scores over b3..b6 4-deep (tag b{3+(i+h)%4}), softmax-denominator
broadcast lb takes the next slot in that rotation, ctx alternates b0/b7.

Scores are transposed s^T[tk,tq] (lhsT=k tile, rhs=q block) with causal
narrowing; matmul cost here is (moving columns) x (cycles/row keyed on the
MOVING operand dtype): fp16 runs 1 cycle/row with no 256-column minimum, so
the r=3 diagonal tile narrows to 128 columns. Softmax denominators come from
element-wise fp16 accumulation of the exp tiles on DVE (2x mode) + ONE
ones-matmul per (head, block) that broadcasts the partition sum - the
per-tile [1,512] ones-matmuls this replaces cost a full 30us of PE. RoPE is
applied in place (half-swap via 2 small SBUF DMAs, sign folded into sinT on
the host; mults split Pool/DVE). Output projection accumulates the 4 heads
in PSUM per 128-token sub-tile; quarter evictions alternate DVE/ACT into an
fp16 [128,2048] staging row, one DMA per sub-tile (per-quarter DMAs on the
last block to shorten the tail). DMAs are batched multi-kt loads; x for block
nb+1 prefetches during attention nb.
"""
import math

import numpy as np

import concourse.bacc as bacc
import concourse.mybir as mybir
import concourse.tile as tile
from concourse.bass_utils import run_bass_kernel_spmd

P = 128           # partitions / head_dim
T = 2048          # context length
C = 2048          # d_model
NKT = C // P      # 16 contraction tiles
NB = T // 512     # 4 token blocks of 512
HPC = 4           # heads per core
NCORES = 8
SCALE = 1.0 / math.sqrt(P)

F32 = mybir.dt.float32
F16 = mybir.dt.float16
EXP = mybir.ActivationFunctionType.Exp
MULT = mybir.AluOpType.mult
ADD = mybir.AluOpType.add

QBANKS = ("b1", "b2", "b0", "b7")   # q-wave accumulators, emission order
KBANKS = ("b3", "b4", "b5", "b6")   # k-wave accumulators
VBANKS = ("b0", "b7", "b1", "b2")   # v-wave accumulators
CTXBANKS = ("b0", "b7")             # ctx_ps alternates by head parity

_CACHE = {}


def _build():
    nc = bacc.Bacc("TRN2", target_bir_lowering=False, debug=False,
                   num_devices=NCORES)
    xg = nc.dram_tensor("xg", (P, NKT, T), F16, kind="ExternalInput").ap()
    wq = nc.dram_tensor("wq", (P, NKT, HPC * P), F16, kind="ExternalInput").ap()
    wk = nc.dram_tensor("wk", (P, NKT, HPC * P), F16, kind="ExternalInput").ap()
    wv = nc.dram_tensor("wv", (P, NKT, HPC * P), F16, kind="ExternalInput").ap()
    wo = nc.dram_tensor("wo", (P, HPC, C), F16, kind="ExternalInput").ap()
    cosT = nc.dram_tensor("cosT", (P, T), F16, kind="ExternalInput").ap()
    sinT = nc.dram_tensor("sinT", (P, T), F16, kind="ExternalInput").ap()
    tri = nc.dram_tensor("tri", (P, P), F16, kind="ExternalInput").ap()
    ones = nc.dram_tensor("ones", (P, P), F16, kind="ExternalInput").ap()
    y = nc.dram_tensor("y", (T, C), F16, kind="ExternalOutput").ap()

    half = P // 2

    with tile.TileContext(nc) as tc:
        with (
            tc.tile_pool(name="gconst", bufs=1) as gpool,
            tc.tile_pool(name="wbuf", bufs=1) as wpool,
            tc.tile_pool(name="xbuf", bufs=1) as xpool,
            tc.tile_pool(name="qkbuf", bufs=1) as qkpool,
            tc.tile_pool(name="vbuf", bufs=1) as vpool,
            tc.tile_pool(name="rope", bufs=1) as rpool,
            tc.tile_pool(name="ptb", bufs=1) as ptpool,
            tc.tile_pool(name="stats", bufs=1) as spool,
            tc.tile_pool(name="ctxb", bufs=1) as cxpool,
            tc.tile_pool(name="yb", bufs=1) as ypool,
            tc.tile_pool(name="ps", bufs=1, space="PSUM") as ps,
        ):
            tri_sb = gpool.tile([P, P], F16, tag="tri")
            ones_sb = gpool.tile([P, P], F16, tag="ones")
            wq_sb = wpool.tile([P, NKT, HPC * P], F16, tag="wq", name="wq_sb")
            wk_sb = wpool.tile([P, NKT, HPC * P], F16, tag="wk", name="wk_sb")
            wv_sb = wpool.tile([P, NKT, HPC * P], F16, tag="wv", name="wv_sb")
            wo_sb = wpool.tile([P, HPC, C], F16, tag="wo", name="wo_sb")
            cos_sb = wpool.tile([P, T], F16, tag="cos", name="cos_sb")
            sin_sb = wpool.tile([P, T], F16, tag="sin", name="sin_sb")

            qk_sb = {}
            for h in range(HPC):
                for part in ("q", "k"):
                    for nb in range(NB):
                        qk_sb[(part, h, nb)] = qkpool.tile(
                            [P, 512], F16, tag=f"{part}{h}n{nb}",
                            name=f"{part}{h}n{nb}_sb")
            v_sb = [vpool.tile([P, 512], F16, tag=f"vb{i}", name=f"v{i}_sb")
                    for i in range(NKT)]

            def wave_accs(banks, label):
                return [ps.tile([P, 512], F32, tag=banks[h],
                                name=f"{label}{h}") for h in range(HPC)]

            def evict_rope_one(nb, part, h, acc):
                nsl = slice(nb * 512, (nb + 1) * 512)
                dst = qk_sb[(part, h, nb)]
                nc.scalar.copy(dst[:], acc[:])
                tmp = rpool.tile([P, 512], F16, tag="rt", bufs=2, name="rtmp")
                nc.sync.dma_start(tmp[0:half, :], dst[half:P, :])
                nc.sync.dma_start(tmp[half:P, :], dst[0:half, :])
                t1 = rpool.tile([P, 512], F16, tag="t1", bufs=2)
                nc.gpsimd.tensor_tensor(t1[:], dst[:], cos_sb[:, nsl], op=MULT)
                t2 = rpool.tile([P, 512], F16, tag="t2", bufs=2)
                nc.vector.tensor_tensor(t2[:], tmp[:], sin_sb[:, nsl], op=MULT)
                nc.vector.tensor_tensor(dst[:], t1[:], t2[:], op=ADD)

            def qk_wave_chain(part, xt, h, bank):
                wsb = wq_sb if part == "q" else wk_sb
                acc = ps.tile([P, 512], F32, tag=bank, name=f"{part}{h}")
                for kt in range(NKT):
                    nc.tensor.matmul(
                        acc[:], wsb[:, kt, h * P:(h + 1) * P],
                        xt[:, kt, :], start=(kt == 0), stop=(kt == NKT - 1))
                return acc

            def v_chain(nb, tt, xt, bank):
                vacc = ps.tile([P, 512], F32, tag=bank, name=f"v{nb}_{tt}")
                for kt in range(NKT):
                    nc.tensor.matmul(
                        vacc[:], xt[:, kt, tt * P:(tt + 1) * P],
                        wv_sb[:, kt, :], start=(kt == 0), stop=(kt == NKT - 1))
                return vacc

            def attention(nb, xt_next, vaccs):
                """vaccs: this block's un-evicted v accumulators (tt 1..3 on
                b1,b2,b7); evictions are emitted just-in-time at the diagonal
                steps of head 0 so ACT serves head 0's first exps first. For
                nb==3 the tt>=1 v chains are emitted inside head 0 as PE
                filler (no next-block waves exist). Head h's softmax stats
                (lb matmul, reciprocal, normalize) are deferred into head
                h+1's pipeline so PE never waits on the DVE denominator
                chain. For nb<3 the next block's q-wave chain for head h is
                emitted right after head h (banks b1/b2 alternating, evicted
                and roped immediately)."""
                nt = 4 * (nb + 1)
                ctx_tiles = {}
                pending = None

                def stats(h, ctx_ps, lacc):
                    # slot (h+3)%4 is the one head h+1 touches last after the
                    # deferred emission point (its i==3), so the reciprocal
                    # drains before the bank is needed again
                    lb = ps.tile([P, 512], F32, tag=f"b{3 + (h + 3) % 4}",
                                 name=f"l{h}_{nb}")
                    nc.tensor.matmul(lb[:], ones_sb[:], lacc[:],
                                     start=True, stop=True)
                    rinv = spool.tile([P, 512], F32, tag="rinv", bufs=2)
                    nc.vector.reciprocal(rinv[:], lb[:])
                    ctx_sb = cxpool.tile([P, 512], F16, tag=f"cx{h}", bufs=2,
                                         name=f"cs{h}_{nb}")
                    nc.vector.tensor_tensor(ctx_sb[:], ctx_ps[:], rinv[:],
                                            op=MULT)
                    ctx_tiles[h] = ctx_sb

                for h in range(HPC):
                    qT = qk_sb[("q", h, nb)]
                    ctx_ps = ps.tile([P, 512], F32, tag=CTXBANKS[h % 2],
                                     name=f"ctx{h}_{nb}")
                    lacc = spool.tile([P, 512], F16, tag="lacc", bufs=2)
                    for i in range(nt):
                        r = i - 4 * nb
                        if h == 0 and nb == NB - 1 and r in (-9, -6, -3):
                            # pull this block's remaining v chains in as filler
                            tt = (r + 9) // 3 + 1
                            vaccs[tt] = v_chain(nb, tt, xt_next,
                                                ("b1", "b2", "b7")[tt - 1])
                        if h == 0 and r >= 1:
                            nc.scalar.copy(v_sb[nb * 4 + r][:], vaccs[r][:])
                        if h > 0 and i == 3 and pending is not None:
                            stats(*pending)
                            pending = None
                        c0 = 0 if r < 1 else r * P
                        osl = slice(c0, 512)
                        kch = qk_sb[("k", h, i // 4)]
                        sps = ps.tile([P, 512], F32, tag=f"b{3 + (i + h) % 4}",
                                      name=f"s{h}_{nb}_{i}")
                        nc.tensor.matmul(
                            sps[:, osl],
                            kch[:, (i % 4) * P:(i % 4 + 1) * P],
                            qT[:, osl], start=True, stop=True)
                        pt = ptpool.tile([P, 512], F16, tag="pt", bufs=6)
                        nc.scalar.activation(pt[:, osl], sps[:, osl], EXP,
                                             scale=SCALE)
                        if r >= 0:
                            dsl = slice(r * P, (r + 1) * P)
                            nc.gpsimd.tensor_tensor(
                                pt[:, dsl], pt[:, dsl], tri_sb[:], op=MULT)
                        nc.tensor.matmul(
                            ctx_ps[:, osl],
                            v_sb[i][:, h * P:(h + 1) * P], pt[:, osl],
                            start=(i == 0), stop=(i == nt - 1))
                        if i == 0:
                            nc.vector.tensor_copy(lacc[:], pt[:])
                        else:
                            nc.vector.tensor_tensor(
                                lacc[:, osl], lacc[:, osl], pt[:, osl], op=ADD)
                    if h == HPC - 1:
                        if pending is not None:
                            stats(*pending)
                            pending = None
                        if nb < NB - 1:
                            qa = qk_wave_chain("q", xt_next, h,
                                               f"b{1 + h % 2}")
                            evict_rope_one(nb + 1, "q", h, qa)
                        stats(h, ctx_ps, lacc)
                    else:
                        pending = (h, ctx_ps, lacc)
                        if nb < NB - 1:
                            # next block's q projection chain as PE filler
                            qa = qk_wave_chain("q", xt_next, h,
                                               f"b{1 + h % 2}")
                            evict_rope_one(nb + 1, "q", h, qa)
                return ctx_tiles

            def outproj(nb, ctx_tiles):
                last = nb == NB - 1
                for sub in range(4):
                    tt = nb * 4 + sub
                    ssl = slice(sub * P, (sub + 1) * P)
                    ysb = ypool.tile([P, C], F16, tag="ysb", bufs=2)
                    for ob in range(NB):
                        obsl = slice(ob * 512, (ob + 1) * 512)
                        yps = ps.tile([P, 512], F32, tag=f"b{1 + ob % 2}",
                                      name=f"y{tt}_{ob}")
                        for h in range(HPC):
                            nc.tensor.matmul(
                                yps[:], ctx_tiles[h][:, ssl],
                                wo_sb[:, h, obsl],
                                start=(h == 0), stop=(h == HPC - 1))
                        if ob % 2 == 0:
                            nc.vector.tensor_copy(ysb[:, obsl], yps[:])
                        else:
                            nc.scalar.copy(ysb[:, obsl], yps[:])
                        if last:
                            nc.sync.dma_start(
                                y[tt * P:(tt + 1) * P, obsl], ysb[:, obsl])
                    if not last:
                        nc.sync.dma_start(y[tt * P:(tt + 1) * P, :], ysb[:])

            # ---- startup DMAs: x(0)+wq+wk interleaved, fine-grained head ----
            x0 = xpool.tile([P, NKT, 512], F16, tag="x", bufs=2, name="x_nb0")
            groups = [(0, 1), (1, 2), (2, 4), (4, 8), (8, 12), (12, 16)]
            for lo, hi in groups:
                gs = slice(lo, hi)
                nc.sync.dma_start(x0[:, gs, :], xg[:, gs, 0:512])
                nc.sync.dma_start(wq_sb[:, gs, :], wq[:, gs, :])
                nc.sync.dma_start(wk_sb[:, gs, :], wk[:, gs, :])
            nc.sync.dma_start(cos_sb[:], cosT)
            nc.sync.dma_start(sin_sb[:], sinT)
            nc.sync.dma_start(wv_sb[:], wv)
            nc.sync.dma_start(tri_sb[:], tri)
            nc.sync.dma_start(ones_sb[:], ones)
            nc.sync.dma_start(wo_sb[:], wo)

            def v_group(nbv, xt_src):
                """tt0 evicted immediately; tt1..3 stay in PSUM for the JIT
                evictions inside attention(nbv) head 0."""
                va0 = v_chain(nbv, 0, xt_src, "b1")
                nc.scalar.copy(v_sb[nbv * 4][:], va0[:])
                return {tt: v_chain(nbv, tt, xt_src, ("b1", "b2", "b7")[tt - 1])
                        for tt in (1, 2, 3)}

            # ---- block 0: plain qk (kt-outer tracks DMA arrival), v ----
            qaccs = wave_accs(QBANKS, "q")
            kaccs = wave_accs(KBANKS, "k")
            for kt in range(NKT):
                for part, wsb, accs in (("q", wq_sb, qaccs),
                                        ("k", wk_sb, kaccs)):
                    for h in range(HPC):
                        nc.tensor.matmul(
                            accs[h][:], wsb[:, kt, h * P:(h + 1) * P],
                            x0[:, kt, :], start=(kt == 0), stop=(kt == NKT - 1))
            for h in range(HPC):
                evict_rope_one(0, "k", h, kaccs[h])
            for h in range(HPC):
                evict_rope_one(0, "q", h, qaccs[h])
            vaccs = v_group(0, x0)

            xt = x0
            for nb in range(NB):
                # prefetch next block's x during attention
                if nb < NB - 1:
                    xt_next = xpool.tile([P, NKT, 512], F16, tag="x", bufs=2,
                                         name=f"x_nb{nb + 1}")
                    nc.sync.dma_start(
                        xt_next[:], xg[:, :, (nb + 1) * 512:(nb + 2) * 512])
                else:
                    xt_next = xt
                ctx_tiles = attention(nb, xt_next, vaccs)
                outproj(nb, ctx_tiles)
                if nb < NB - 1:
                    for h in range(HPC):
                        ka = qk_wave_chain("k", xt_next, h, f"b{1 + h % 2}")
                        evict_rope_one(nb + 1, "k", h, ka)
                    if nb + 1 < NB - 1:
                        vaccs = v_group(nb + 1, xt_next)
                    else:
                        va0 = v_chain(nb + 1, 0, xt_next, "b1")
                        nc.scalar.copy(v_sb[(nb + 1) * 4][:], va0[:])
                        vaccs = {}
                xt = xt_next

    nc.compile()
    return nc


def _build_kernel():
    if "k" not in _CACHE:
        _CACHE["k"] = _build()
    return _CACHE["k"]


def prepare_in_maps(x, W_qkv, W_o, cos, sin):
    f16 = np.float16
    tri01 = (np.arange(P)[:, None] <= np.arange(P)[None, :]).astype(f16)
    ones = np.ones((P, P), dtype=f16)
    cosT = np.ascontiguousarray(cos.T).astype(f16)
    # rotate_half sign folded in: rows (head dims) 0..63 negated
    sgn = np.where(np.arange(P) < P // 2, -1.0, 1.0).astype(np.float32)
    sinT = (sin.T * sgn[:, None]).astype(f16)

    in_maps = []
    for core in range(NCORES):
        b = core // 4
        hg0 = (core % 4) * HPC
        rows = slice(hg0 * P, (hg0 + HPC) * P)
        xT = x[b].T  # [C, T]
        xg = np.ascontiguousarray(
            xT.reshape(NKT, P, T).transpose(1, 0, 2)).astype(f16)

        def wprep(w):  # [512 rows, 2048 c] -> [P, NKT, 512]
            return np.ascontiguousarray(
                w.T.reshape(NKT, P, HPC * P).transpose(1, 0, 2)).astype(f16)

        wq_t = wprep(W_qkv[0 * C:1 * C][rows])
        wk_t = wprep(W_qkv[1 * C:2 * C][rows])
        wv_t = wprep(W_qkv[2 * C:3 * C][rows])
        wo_t = np.ascontiguousarray(
            W_o[:, rows].T.reshape(HPC, P, C).transpose(1, 0, 2)).astype(f16)
        in_maps.append({
            "xg": xg, "wq": wq_t, "wk": wk_t, "wv": wv_t, "wo": wo_t,
            "cosT": cosT, "sinT": sinT, "tri": tri01, "ones": ones,
        })
    return in_maps


def gather(results, b_o):
    y = np.zeros((2, T, C), dtype=np.float32)
    for core in range(NCORES):
        y[core // 4] += results[core]["y"].astype(np.float32)
    y += np.asarray(b_o, dtype=np.float32)[None, None, :]
    return y


def kernel(x, W_qkv, W_o, b_o, cos, sin):
    x = np.asarray(x, dtype=np.float32)
    W_qkv = np.asarray(W_qkv, dtype=np.float32)
    W_o = np.asarray(W_o, dtype=np.float32)
    cos = np.asarray(cos, dtype=np.float32)
    sin = np.asarray(sin, dtype=np.float32)
    nc = _build_kernel()
    in_maps = prepare_in_maps(x, W_qkv, W_o, cos, sin)
    res = run_bass_kernel_spmd(nc, in_maps, core_ids=list(range(NCORES)))
    return gather(res.results, b_o)


# revision 14
# speedup vs baseline: 1.2729x; 1.0074x over previous
"""Causal multi-head attention (RoPE) on 8 TRN2 NeuronCores.

Problem: x[2,2048,2048] -> qkv proj -> rope -> causal attention (16 heads,
head_dim 128) -> output proj + bias. Sharding: (batch, head-group) across the
8 cores - core c handles batch c//4 and heads 4*(c%4)..4*(c%4)+3. Each core
computes a partial output projection over its heads' channels; the host sums
the 4 partials per batch and adds b_o.

Single-pass token-outer pipeline, everything fp16 on device (PSUM accumulation
stays f32; final host reduction in f32; validated rel err ~5e-4 vs the fp32
reference). The exp throughput on ACT (0.833ns/col) exactly matches the
scores+AV cost on PE, so attention phases are ACT-bound unless PE borrows
other work: the next block's QKV projection is software-pipelined INTO the
attention window as three waves:

    attn(nb) heads -> q-wave(nb+1) -> outproj(nb) -> k-wave(nb+1)
                   -> v-wave(nb+1) -> attn(nb+1) ...

Waves are accumulator-major (16 kt matmuls per PSUM bank) with a bank map
chosen so each wave's first banks were freed earliest by the previous phase:
q-wave on b1,b2,b0,b7 / k-wave on b3..b6 / v-wave on b0,b7,b1,b2; attention
rotates scores over b3..b6 4-deep (tag b{3+(i+h)%4}), softmax-denominator
broadcast lb takes the next slot in that rotation, ctx alternates b0/b7.

Scores are transposed s^T[tk,tq] (lhsT=k tile, rhs=q block) with causal
narrowing; matmul cost here is (moving columns) x (cycles/row keyed on the
MOVING operand dtype): fp16 runs 1 cycle/row with no 256-column minimum, so
the r=3 diagonal tile narrows to 128 columns. Softmax denominators come from
element-wise fp16 accumulation of the exp tiles on DVE (2x mode) + ONE
ones-matmul per (head, block) that broadcasts the partition sum - the
per-tile [1,512] ones-matmuls this replaces cost a full 30us of PE. RoPE is
applied in place (half-swap via 2 small SBUF DMAs, sign folded into sinT on
the host; mults split Pool/DVE). Output projection accumulates the 4 heads
in PSUM per 128-token sub-tile; quarter evictions alternate DVE/ACT into an
fp16 [128,2048] staging row, one DMA per sub-tile (per-quarter DMAs on the
last block to shorten the tail). DMAs are batched multi-kt loads; x for block
nb+1 prefetches during attention nb.
"""
import math

import numpy as np

import concourse.bacc as bacc
import concourse.mybir as mybir
import concourse.tile as tile
from concourse.bass_utils import run_bass_kernel_spmd

P = 128           # partitions / head_dim
T = 2048          # context length
C = 2048          # d_model
NKT = C // P      # 16 contraction tiles
NB = T // 512     # 4 token blocks of 512
HPC = 4           # heads per core
NCORES = 8
SCALE = 1.0 / math.sqrt(P)

F32 = mybir.dt.float32
F16 = mybir.dt.float16
EXP = mybir.ActivationFunctionType.Exp
MULT = mybir.AluOpType.mult
ADD = mybir.AluOpType.add

QBANKS = ("b1", "b2", "b0", "b7")   # q-wave accumulators, emission order
KBANKS = ("b3", "b4", "b5", "b6")   # k-wave accumulators
VBANKS = ("b0", "b7", "b1", "b2")   # v-wave accumulators
CTXBANKS = ("b0", "b7")             # ctx_ps alternates by head parity

_CACHE = {}


def _build():
    nc = bacc.Bacc("TRN2", target_bir_lowering=False, debug=False,
                   num_devices=NCORES)
    xg = nc.dram_tensor("xg", (P, NKT, T), F16, kind="ExternalInput").ap()
    wq = nc.dram_tensor("wq", (P, NKT, HPC * P), F16, kind="ExternalInput").ap()
    wk = nc.dram_tensor("wk", (P, NKT, HPC * P), F16, kind="ExternalInput").ap()
    wv = nc.dram_tensor("wv", (P, NKT, HPC * P), F16, kind="ExternalInput").ap()
    wo = nc.dram_tensor("wo", (P, HPC, C), F16, kind="ExternalInput").ap()
    cosT = nc.dram_tensor("cosT", (P, T), F16, kind="ExternalInput").ap()
    sinT = nc.dram_tensor("sinT", (P, T), F16, kind="ExternalInput").ap()
    tri = nc.dram_tensor("tri", (P, P), F16, kind="ExternalInput").ap()
    ones = nc.dram_tensor("ones", (P, P), F16, kind="ExternalInput").ap()
    y = nc.dram_tensor("y", (T, C), F16, kind="ExternalOutput").ap()

    half = P // 2

    with tile.TileContext(nc) as tc:
        with (
            tc.tile_pool(name="gconst", bufs=1) as gpool,
            tc.tile_pool(name="wbuf", bufs=1) as wpool,
            tc.tile_pool(name="xbuf", bufs=1) as xpool,
            tc.tile_pool(name="qkbuf", bufs=1) as qkpool,
            tc.tile_pool(name="vbuf", bufs=1) as vpool,
            tc.tile_pool(name="rope", bufs=1) as rpool,
            tc.tile_pool(name="ptb", bufs=1) as ptpool,
            tc.tile_pool(name="stats", bufs=1) as spool,
            tc.tile_pool(name="ctxb", bufs=1) as cxpool,
            tc.tile_pool(name="yb", bufs=1) as ypool,
            tc.tile_pool(name="ps", bufs=1, space="PSUM") as ps,
        ):
            tri_sb = gpool.tile([P, P], F16, tag="tri")
            ones_sb = gpool.tile([P, P], F16, tag="ones")
            wq_sb = wpool.tile([P, NKT, HPC * P], F16, tag="wq", name="wq_sb")
            wk_sb = wpool.tile([P, NKT, HPC * P], F16, tag="wk", name="wk_sb")
            wv_sb = wpool.tile([P, NKT, HPC * P], F16, tag="wv", name="wv_sb")
            wo_sb = wpool.tile([P, HPC, C], F16, tag="wo", name="wo_sb")
            cos_sb = wpool.tile([P, T], F16, tag="cos", name="cos_sb")
            sin_sb = wpool.tile([P, T], F16, tag="sin", name="sin_sb")

            qk_sb = {}
            for h in range(HPC):
                for part in ("q", "k"):
                    for nb in range(NB):
                        qk_sb[(part, h, nb)] = qkpool.tile(
                            [P, 512], F16, tag=f"{part}{h}n{nb}",
                            name=f"{part}{h}n{nb}_sb")
            v_sb = [vpool.tile([P, 512], F16, tag=f"vb{i}", name=f"v{i}_sb")
                    for i in range(NKT)]

            def wave_accs(banks, label):
                return [ps.tile([P, 512], F32, tag=banks[h],
                                name=f"{label}{h}") for h in range(HPC)]

            def rope_one(nb, part, h):
                """In-place rope on an evicted q/k chunk. Emitted away from
                the attention masks: the Pool t1 multiply is 1.1us, and a
                diagonal mask queued behind it stalls the AV matmuls."""
                nsl = slice(nb * 512, (nb + 1) * 512)
                dst = qk_sb[(part, h, nb)]
                tmp = rpool.tile([P, 512], F16, tag="rt", bufs=2, name="rtmp")
                nc.sync.dma_start(tmp[0:half, :], dst[half:P, :])
                nc.sync.dma_start(tmp[half:P, :], dst[0:half, :])
                t1 = rpool.tile([P, 512], F16, tag="t1", bufs=2)
                nc.gpsimd.tensor_tensor(t1[:], dst[:], cos_sb[:, nsl], op=MULT)
                t2 = rpool.tile([P, 512], F16, tag="t2", bufs=2)
                nc.vector.tensor_tensor(t2[:], tmp[:], sin_sb[:, nsl], op=MULT)
                nc.vector.tensor_tensor(dst[:], t1[:], t2[:], op=ADD)

            def evict_rope_one(nb, part, h, acc):
                nc.scalar.copy(qk_sb[(part, h, nb)][:], acc[:])
                rope_one(nb, part, h)

            def qk_wave_chain(part, xt, h, bank):
                wsb = wq_sb if part == "q" else wk_sb
                acc = ps.tile([P, 512], F32, tag=bank, name=f"{part}{h}")
                for kt in range(NKT):
                    nc.tensor.matmul(
                        acc[:], wsb[:, kt, h * P:(h + 1) * P],
                        xt[:, kt, :], start=(kt == 0), stop=(kt == NKT - 1))
                return acc

            def v_chain(nb, tt, xt, bank):
                vacc = ps.tile([P, 512], F32, tag=bank, name=f"v{nb}_{tt}")
                for kt in range(NKT):
                    nc.tensor.matmul(
                        vacc[:], xt[:, kt, tt * P:(tt + 1) * P],
                        wv_sb[:, kt, :], start=(kt == 0), stop=(kt == NKT - 1))
                return vacc

            def attention(nb, xt_next, vaccs):
                """vaccs: this block's un-evicted v accumulators (tt 1..3 on
                b1,b2,b7); evictions are emitted just-in-time at the diagonal
                steps of head 0 so ACT serves head 0's first exps first. For
                nb==3 the tt>=1 v chains are emitted inside head 0 as PE
                filler (no next-block waves exist). Head h's softmax stats
                (lb matmul, reciprocal, normalize) are deferred into head
                h+1's pipeline so PE never waits on the DVE denominator
                chain. For nb<3 the next block's q-wave chain for head h is
                emitted right after head h (banks b1/b2 alternating, evicted
                and roped immediately)."""
                nt = 4 * (nb + 1)
                ctx_tiles = {}
                pending = None

                def stats(h, ctx_ps, lacc):
                    # slot (h+3)%4 is the one head h+1 touches last after the
                    # deferred emission point (its i==3), so the reciprocal
                    # drains before the bank is needed again
                    lb = ps.tile([P, 512], F32, tag=f"b{3 + (h + 3) % 4}",
                                 name=f"l{h}_{nb}")
                    nc.tensor.matmul(lb[:], ones_sb[:], lacc[:],
                                     start=True, stop=True)
                    rinv = spool.tile([P, 512], F32, tag="rinv", bufs=2)
                    ctx_sb = cxpool.tile([P, 512], F16, tag=f"cx{h}", bufs=2,
                                         name=f"cs{h}_{nb}")
                    # last head's normalize gates the output projection: do it
                    # in halves so outproj's first sub-tile unblocks early
                    for lo, hi in ((0, 256), (256, 512)) if h == HPC - 1 \
                            else ((0, 512),):
                        csl = slice(lo, hi)
                        nc.vector.reciprocal(rinv[:, csl], lb[:, csl])
                        nc.vector.tensor_tensor(ctx_sb[:, csl],
                                                ctx_ps[:, csl],
                                                rinv[:, csl], op=MULT)
                    ctx_tiles[h] = ctx_sb

                for h in range(HPC):
                    qT = qk_sb[("q", h, nb)]
                    ctx_ps = ps.tile([P, 512], F32, tag=CTXBANKS[h % 2],
                                     name=f"ctx{h}_{nb}")
                    lacc = spool.tile([P, 512], F16, tag="lacc", bufs=2)
                    for i in range(nt):
                        r = i - 4 * nb
                        if h == 0 and nb == NB - 1 and r in (-9, -6, -3):
                            # pull this block's remaining v chains in as filler
                            tt = (r + 9) // 3 + 1
                            vaccs[tt] = v_chain(nb, tt, xt_next,
                                                ("b1", "b2", "b7")[tt - 1])
                        if h == 0 and r >= 1:
                            nc.scalar.copy(v_sb[nb * 4 + r][:], vaccs[r][:])
                        if h > 0 and i == 3 and pending is not None:
                            stats(*pending)
                            pending = None
                        c0 = 0 if r < 1 else r * P
                        osl = slice(c0, 512)
                        kch = qk_sb[("k", h, i // 4)]
                        sps = ps.tile([P, 512], F32, tag=f"b{3 + (i + h) % 4}",
                                      name=f"s{h}_{nb}_{i}")
                        nc.tensor.matmul(
                            sps[:, osl],
                            kch[:, (i % 4) * P:(i % 4 + 1) * P],
                            qT[:, osl], start=True, stop=True)
                        pt = ptpool.tile([P, 512], F16, tag="pt", bufs=6)
                        nc.scalar.activation(pt[:, osl], sps[:, osl], EXP,
                                             scale=SCALE)
                        if r >= 0:
                            dsl = slice(r * P, (r + 1) * P)
                            nc.gpsimd.tensor_tensor(
                                pt[:, dsl], pt[:, dsl], tri_sb[:], op=MULT)
                        nc.tensor.matmul(
                            ctx_ps[:, osl],
                            v_sb[i][:, h * P:(h + 1) * P], pt[:, osl],
                            start=(i == 0), stop=(i == nt - 1))
                        if i == 0:
                            nc.vector.tensor_copy(lacc[:], pt[:])
                        else:
                            nc.vector.tensor_tensor(
                                lacc[:, osl], lacc[:, osl], pt[:, osl], op=ADD)
                    if h == HPC - 1:
                        if pending is not None:
                            stats(*pending)
                            pending = None
                        if nb < NB - 1:
                            qa = qk_wave_chain("q", xt_next, h,
                                               f"b{1 + h % 2}")
                            nc.scalar.copy(qk_sb[("q", h, nb + 1)][:], qa[:])
                        stats(h, ctx_ps, lacc)
                    else:
                        pending = (h, ctx_ps, lacc)
                        if nb < NB - 1:
                            # next block's q projection chain as PE filler;
                            # rope deferred past the masks (see rope_one)
                            qa = qk_wave_chain("q", xt_next, h,
                                               f"b{1 + h % 2}")
                            nc.scalar.copy(qk_sb[("q", h, nb + 1)][:], qa[:])
                if nb < NB - 1:
                    for h in range(HPC):
                        rope_one(nb + 1, "q", h)
                return ctx_tiles

            def outproj(nb, ctx_tiles):
                last = nb == NB - 1
                for sub in range(4):
                    tt = nb * 4 + sub
                    ssl = slice(sub * P, (sub + 1) * P)
                    ysb = ypool.tile([P, C], F16, tag="ysb", bufs=2)
                    for ob in range(NB):
                        obsl = slice(ob * 512, (ob + 1) * 512)
                        # 3-bank ring (b1,b2,b3,b2) absorbs eviction latency;
                        # b1 frees earliest for the k-wave that follows
                        yps = ps.tile([P, 512], F32,
                                      tag=("b1", "b2", "b3", "b2")[ob],
                                      name=f"y{tt}_{ob}")
                        for h in range(HPC):
                            nc.tensor.matmul(
                                yps[:], ctx_tiles[h][:, ssl],
                                wo_sb[:, h, obsl],
                                start=(h == 0), stop=(h == HPC - 1))
                        if ob % 2 == 0:
                            nc.vector.tensor_copy(ysb[:, obsl], yps[:])
                        else:
                            nc.scalar.copy(ysb[:, obsl], yps[:])
                        if last:
                            nc.sync.dma_start(
                                y[tt * P:(tt + 1) * P, obsl], ysb[:, obsl])
                    if not last:
                        nc.sync.dma_start(y[tt * P:(tt + 1) * P, :], ysb[:])

            # ---- startup DMAs: x(0)+wq+wk interleaved, fine-grained head ----
            x0 = xpool.tile([P, NKT, 512], F16, tag="x", bufs=2, name="x_nb0")
            groups = [(0, 1), (1, 2), (2, 4), (4, 8), (8, 12), (12, 16)]
            for lo, hi in groups:
                gs = slice(lo, hi)
                nc.sync.dma_start(x0[:, gs, :], xg[:, gs, 0:512])
                nc.sync.dma_start(wq_sb[:, gs, :], wq[:, gs, :])
                nc.sync.dma_start(wk_sb[:, gs, :], wk[:, gs, :])
            nc.sync.dma_start(cos_sb[:], cosT)
            nc.sync.dma_start(sin_sb[:], sinT)
            nc.sync.dma_start(wv_sb[:], wv)
            nc.sync.dma_start(tri_sb[:], tri)
            nc.sync.dma_start(ones_sb[:], ones)
            nc.sync.dma_start(wo_sb[:], wo)

            def v_group(nbv, xt_src):
                """tt0 evicted immediately; tt1..3 stay in PSUM for the JIT
                evictions inside attention(nbv) head 0."""
                va0 = v_chain(nbv, 0, xt_src, "b1")
                nc.scalar.copy(v_sb[nbv * 4][:], va0[:])
                return {tt: v_chain(nbv, tt, xt_src, ("b1", "b2", "b7")[tt - 1])
                        for tt in (1, 2, 3)}

            # ---- block 0: plain qk (kt-outer tracks DMA arrival), v ----
            qaccs = wave_accs(QBANKS, "q")
            kaccs = wave_accs(KBANKS, "k")
            for kt in range(NKT):
                for part, wsb, accs in (("q", wq_sb, qaccs),
                                        ("k", wk_sb, kaccs)):
                    for h in range(HPC):
                        nc.tensor.matmul(
                            accs[h][:], wsb[:, kt, h * P:(h + 1) * P],
                            x0[:, kt, :], start=(kt == 0), stop=(kt == NKT - 1))
            for h in range(HPC):
                evict_rope_one(0, "k", h, kaccs[h])
            for h in range(HPC):
                evict_rope_one(0, "q", h, qaccs[h])
            vaccs = v_group(0, x0)

            xt = x0
            for nb in range(NB):
                # prefetch next block's x during attention
                if nb < NB - 1:
                    xt_next = xpool.tile([P, NKT, 512], F16, tag="x", bufs=2,
                                         name=f"x_nb{nb + 1}")
                    nc.sync.dma_start(
                        xt_next[:], xg[:, :, (nb + 1) * 512:(nb + 2) * 512])
                else:
                    xt_next = xt
                ctx_tiles = attention(nb, xt_next, vaccs)
                outproj(nb, ctx_tiles)
                if nb < NB - 1:
                    for h in range(HPC):
                        ka = qk_wave_chain("k", xt_next, h, f"b{1 + h % 2}")
                        evict_rope_one(nb + 1, "k", h, ka)
                    if nb + 1 < NB - 1:
                        vaccs = v_group(nb + 1, xt_next)
                    else:
                        va0 = v_chain(nb + 1, 0, xt_next, "b1")
                        nc.scalar.copy(v_sb[(nb + 1) * 4][:], va0[:])
                        vaccs = {}
                xt = xt_next

    nc.compile()
    return nc


def _build_kernel():
    if "k" not in _CACHE:
        _CACHE["k"] = _build()
    return _CACHE["k"]


def prepare_in_maps(x, W_qkv, W_o, cos, sin):
    f16 = np.float16
    tri01 = (np.arange(P)[:, None] <= np.arange(P)[None, :]).astype(f16)
    ones = np.ones((P, P), dtype=f16)
    cosT = np.ascontiguousarray(cos.T).astype(f16)
    # rotate_half sign folded in: rows (head dims) 0..63 negated
    sgn = np.where(np.arange(P) < P // 2, -1.0, 1.0).astype(np.float32)
    sinT = (sin.T * sgn[:, None]).astype(f16)

    in_maps = []
    for core in range(NCORES):
        b = core // 4
        hg0 = (core % 4) * HPC
        rows = slice(hg0 * P, (hg0 + HPC) * P)
        xT = x[b].T  # [C, T]
        xg = np.ascontiguousarray(
            xT.reshape(NKT, P, T).transpose(1, 0, 2)).astype(f16)

        def wprep(w):  # [512 rows, 2048 c] -> [P, NKT, 512]
            return np.ascontiguousarray(
                w.T.reshape(NKT, P, HPC * P).transpose(1, 0, 2)).astype(f16)

        wq_t = wprep(W_qkv[0 * C:1 * C][rows])
        wk_t = wprep(W_qkv[1 * C:2 * C][rows])
        wv_t = wprep(W_qkv[2 * C:3 * C][rows])
        wo_t = np.ascontiguousarray(
            W_o[:, rows].T.reshape(HPC, P, C).transpose(1, 0, 2)).astype(f16)
        in_maps.append({
            "xg": xg, "wq": wq_t, "wk": wk_t, "wv": wv_t, "wo": wo_t,
            "cosT": cosT, "sinT": sinT, "tri": tri01, "ones": ones,
        })
    return in_maps


def gather(results, b_o):
    y = np.zeros((2, T, C), dtype=np.float32)
    for core in range(NCORES):
        y[core // 4] += results[core]["y"].astype(np.float32)
    y += np.asarray(b_o, dtype=np.float32)[None, None, :]
    return y


def kernel(x, W_qkv, W_o, b_o, cos, sin):
    x = np.asarray(x, dtype=np.float32)
    W_qkv = np.asarray(W_qkv, dtype=np.float32)
    W_o = np.asarray(W_o, dtype=np.float32)
    cos = np.asarray(cos, dtype=np.float32)
    sin = np.asarray(sin, dtype=np.float32)
    nc = _build_kernel()
    in_maps = prepare_in_maps(x, W_qkv, W_o, cos, sin)
    res = run_bass_kernel_spmd(nc, in_maps, core_ids=list(range(NCORES)))
    return gather(res.results, b_o)


# revision 18
# speedup vs baseline: 1.2929x; 1.0157x over previous
"""Causal multi-head attention (RoPE) on 8 TRN2 NeuronCores.

Problem: x[2,2048,2048] -> qkv proj -> rope -> causal attention (16 heads,
head_dim 128) -> output proj + bias. Sharding: (batch, head-group) across the
8 cores - core c handles batch c//4 and heads 4*(c%4)..4*(c%4)+3. Each core
computes a partial output projection over its heads' channels; the host sums
the 4 partials per batch and adds b_o.

Single-pass token-outer pipeline, everything fp16 on device (PSUM accumulation
stays f32; final host reduction in f32; validated rel err ~5e-4 vs the fp32
reference). The exp throughput on ACT (0.833ns/col) exactly matches the
scores+AV cost on PE, so attention phases are ACT-bound unless PE borrows
other work: the next block's QKV projection is software-pipelined INTO the
attention window as three waves:

    attn(nb) heads -> q-wave(nb+1) -> outproj(nb) -> k-wave(nb+1)
                   -> v-wave(nb+1) -> attn(nb+1) ...

Waves are accumulator-major (16 kt matmuls per PSUM bank) with a bank map
chosen so each wave's first banks were freed earliest by the previous phase:
q-wave on b1,b2,b0,b7 / k-wave on b3..b6 / v-wave on b0,b7,b1,b2; attention
rotates scores over b3..b6 4-deep (tag b{3+(i+h)%4}), softmax-denominator
broadcast lb takes the next slot in that rotation, ctx alternates b0/b7.

Scores are transposed s^T[tk,tq] (lhsT=k tile, rhs=q block) with causal
narrowing; matmul cost here is (moving columns) x (cycles/row keyed on the
MOVING operand dtype): fp16 runs 1 cycle/row with no 256-column minimum, so
the r=3 diagonal tile narrows to 128 columns. Softmax denominators come from
element-wise fp16 accumulation of the exp tiles on DVE (2x mode) + ONE
ones-matmul per (head, block) that broadcasts the partition sum - the
per-tile [1,512] ones-matmuls this replaces cost a full 30us of PE. RoPE is
applied in place (half-swap via 2 small SBUF DMAs, sign folded into sinT on
the host; mults split Pool/DVE). Output projection accumulates the 4 heads
in PSUM per 128-token sub-tile; quarter evictions alternate DVE/ACT into an
fp16 [128,2048] staging row, one DMA per sub-tile (per-quarter DMAs on the
last block to shorten the tail). DMAs are batched multi-kt loads; x for block
nb+1 prefetches during attention nb.
"""
import math

import numpy as np

import concourse.bacc as bacc
import concourse.mybir as mybir
import concourse.tile as tile
from concourse.bass_utils import run_bass_kernel_spmd

P = 128           # partitions / head_dim
T = 2048          # context length
C = 2048          # d_model
NKT = C // P      # 16 contraction tiles
NB = T // 512     # 4 token blocks of 512
HPC = 4           # heads per core
NCORES = 8
SCALE = 1.0 / math.sqrt(P)

F32 = mybir.dt.float32
F16 = mybir.dt.float16
EXP = mybir.ActivationFunctionType.Exp
MULT = mybir.AluOpType.mult
ADD = mybir.AluOpType.add

QBANKS = ("b1", "b2", "b0", "b7")   # q-wave accumulators, emission order
KBANKS = ("b3", "b4", "b5", "b6")   # k-wave accumulators
VBANKS = ("b0", "b7", "b1", "b2")   # v-wave accumulators
CTXBANKS = ("b0", "b7")             # ctx_ps alternates by head parity

_CACHE = {}


def _build():
    nc = bacc.Bacc("TRN2", target_bir_lowering=False, debug=False,
                   num_devices=NCORES)
    xg = nc.dram_tensor("xg", (P, NKT, T), F16, kind="ExternalInput").ap()
    wq = nc.dram_tensor("wq", (P, NKT, HPC * P), F16, kind="ExternalInput").ap()
    wk = nc.dram_tensor("wk", (P, NKT, HPC * P), F16, kind="ExternalInput").ap()
    wv = nc.dram_tensor("wv", (P, NKT, HPC * P), F16, kind="ExternalInput").ap()
    wo = nc.dram_tensor("wo", (P, HPC, C), F16, kind="ExternalInput").ap()
    cosT = nc.dram_tensor("cosT", (P, T), F16, kind="ExternalInput").ap()
    sinT = nc.dram_tensor("sinT", (P, T), F16, kind="ExternalInput").ap()
    tri = nc.dram_tensor("tri", (P, P), F16, kind="ExternalInput").ap()
    ones = nc.dram_tensor("ones", (P, P), F16, kind="ExternalInput").ap()
    y = nc.dram_tensor("y", (T, C), F16, kind="ExternalOutput").ap()

    half = P // 2

    with tile.TileContext(nc) as tc:
        with (
            tc.tile_pool(name="gconst", bufs=1) as gpool,
            tc.tile_pool(name="wbuf", bufs=1) as wpool,
            tc.tile_pool(name="xbuf", bufs=1) as xpool,
            tc.tile_pool(name="qkbuf", bufs=1) as qkpool,
            tc.tile_pool(name="vbuf", bufs=1) as vpool,
            tc.tile_pool(name="rope", bufs=1) as rpool,
            tc.tile_pool(name="ptb", bufs=1) as ptpool,
            tc.tile_pool(name="stats", bufs=1) as spool,
            tc.tile_pool(name="ctxb", bufs=1) as cxpool,
            tc.tile_pool(name="yb", bufs=1) as ypool,
            tc.tile_pool(name="ps", bufs=1, space="PSUM") as ps,
        ):
            tri_sb = gpool.tile([P, P], F16, tag="tri")
            ones_sb = gpool.tile([P, P], F16, tag="ones")
            wq_sb = wpool.tile([P, NKT, HPC * P], F16, tag="wq", name="wq_sb")
            wk_sb = wpool.tile([P, NKT, HPC * P], F16, tag="wk", name="wk_sb")
            wv_sb = wpool.tile([P, NKT, HPC * P], F16, tag="wv", name="wv_sb")
            wo_sb = wpool.tile([P, HPC, C], F16, tag="wo", name="wo_sb")
            cos_sb = wpool.tile([P, T], F16, tag="cos", name="cos_sb")
            sin_sb = wpool.tile([P, T], F16, tag="sin", name="sin_sb")

            qk_sb = {}
            for h in range(HPC):
                for part in ("q", "k"):
                    for nb in range(NB):
                        qk_sb[(part, h, nb)] = qkpool.tile(
                            [P, 512], F16, tag=f"{part}{h}n{nb}",
                            name=f"{part}{h}n{nb}_sb")
            v_sb = [vpool.tile([P, 512], F16, tag=f"vb{i}", name=f"v{i}_sb")
                    for i in range(NKT)]

            def wave_accs(banks, label):
                return [ps.tile([P, 512], F32, tag=banks[h],
                                name=f"{label}{h}") for h in range(HPC)]

            def rope_one(nb, part, h):
                """In-place rope on an evicted q/k chunk. Emitted away from
                the attention masks: the Pool t1 multiply is 1.1us, and a
                diagonal mask queued behind it stalls the AV matmuls."""
                nsl = slice(nb * 512, (nb + 1) * 512)
                dst = qk_sb[(part, h, nb)]
                tmp = rpool.tile([P, 512], F16, tag="rt", bufs=2, name="rtmp")
                nc.sync.dma_start(tmp[0:half, :], dst[half:P, :])
                nc.sync.dma_start(tmp[half:P, :], dst[0:half, :])
                t1 = rpool.tile([P, 512], F16, tag="t1", bufs=2)
                nc.gpsimd.tensor_tensor(t1[:], dst[:], cos_sb[:, nsl], op=MULT)
                t2 = rpool.tile([P, 512], F16, tag="t2", bufs=2)
                nc.vector.tensor_tensor(t2[:], tmp[:], sin_sb[:, nsl], op=MULT)
                nc.vector.tensor_tensor(dst[:], t1[:], t2[:], op=ADD)

            def evict_rope_one(nb, part, h, acc):
                nc.scalar.copy(qk_sb[(part, h, nb)][:], acc[:])
                rope_one(nb, part, h)

            def qk_wave_chain(part, xt, h, bank):
                wsb = wq_sb if part == "q" else wk_sb
                acc = ps.tile([P, 512], F32, tag=bank, name=f"{part}{h}")
                for kt in range(NKT):
                    nc.tensor.matmul(
                        acc[:], wsb[:, kt, h * P:(h + 1) * P],
                        xt[:, kt, :], start=(kt == 0), stop=(kt == NKT - 1))
                return acc

            def v_chain(nb, tt, xt, bank):
                vacc = ps.tile([P, 512], F32, tag=bank, name=f"v{nb}_{tt}")
                for kt in range(NKT):
                    nc.tensor.matmul(
                        vacc[:], xt[:, kt, tt * P:(tt + 1) * P],
                        wv_sb[:, kt, :], start=(kt == 0), stop=(kt == NKT - 1))
                return vacc

            def attention(nb, xt_next, vaccs):
                """vaccs: this block's un-evicted v accumulators (tt 1..3 on
                b1,b2,b7); evictions are emitted just-in-time at the diagonal
                steps of head 0 so ACT serves head 0's first exps first. For
                nb==3 the tt>=1 v chains are emitted inside head 0 as PE
                filler (no next-block waves exist). Head h's softmax stats
                (lb matmul, reciprocal, normalize) are deferred into head
                h+1's pipeline so PE never waits on the DVE denominator
                chain. For nb<3 the next block's q-wave chain for head h is
                emitted right after head h (banks b1/b2 alternating, evicted
                and roped immediately)."""
                nt = 4 * (nb + 1)
                ctx_tiles = {}
                pending = None

                def q_filler():
                    # next block's q projection, one matmul per drain unit,
                    # each chain evicted (ACT) as soon as it completes
                    for fh in range(HPC):
                        acc = ps.tile([P, 512], F32, tag=f"b{1 + fh % 2}",
                                      name=f"q{fh}")
                        for kt in range(NKT):
                            nc.tensor.matmul(
                                acc[:], wq_sb[:, kt, fh * P:(fh + 1) * P],
                                xt_next[:, kt, :], start=(kt == 0),
                                stop=(kt == NKT - 1))
                            yield
                        nc.scalar.copy(qk_sb[("q", fh, nb + 1)][:], acc[:])

                filler = q_filler() if nb < NB - 1 else None

                def drain(n):
                    if filler is None:
                        return
                    for _ in range(n):
                        if next(filler, "done") == "done":
                            break

                per_step = max(1, 58 // (3 * nt))

                def stats(h, ctx_ps, lacc):
                    # slot (h+3)%4 is the one head h+1 touches last after the
                    # deferred emission point (its i==3), so the reciprocal
                    # drains before the bank is needed again
                    lb = ps.tile([P, 512], F32, tag=f"b{3 + (h + 3) % 4}",
                                 name=f"l{h}_{nb}")
                    nc.tensor.matmul(lb[:], ones_sb[:], lacc[:],
                                     start=True, stop=True)
                    rinv = spool.tile([P, 512], F32, tag="rinv", bufs=2)
                    ctx_sb = cxpool.tile([P, 512], F16, tag=f"cx{h}", bufs=2,
                                         name=f"cs{h}_{nb}")
                    # last head's normalize gates the output projection: do it
                    # in halves so outproj's first sub-tile unblocks early
                    for lo, hi in ((0, 256), (256, 512)) if h == HPC - 1 \
                            else ((0, 512),):
                        csl = slice(lo, hi)
                        nc.vector.reciprocal(rinv[:, csl], lb[:, csl])
                        nc.vector.tensor_tensor(ctx_sb[:, csl],
                                                ctx_ps[:, csl],
                                                rinv[:, csl], op=MULT)
                    ctx_tiles[h] = ctx_sb

                for h in range(HPC):
                    qT = qk_sb[("q", h, nb)]
                    ctx_ps = ps.tile([P, 512], F32, tag=CTXBANKS[h % 2],
                                     name=f"ctx{h}_{nb}")
                    lacc = spool.tile([P, 512], F16, tag="lacc", bufs=2)
                    for i in range(nt):
                        r = i - 4 * nb
                        if h == 0 and nb == NB - 1 and r in (-9, -6, -3):
                            # pull this block's remaining v chains in as filler
                            tt = (r + 9) // 3 + 1
                            vaccs[tt] = v_chain(nb, tt, xt_next,
                                                ("b1", "b2", "b7")[tt - 1])
                        if h == 0 and r >= 1:
                            nc.scalar.copy(v_sb[nb * 4 + r][:], vaccs[r][:])
                        if h > 0 and i == 3 and pending is not None:
                            stats(*pending)
                            pending = None
                        c0 = 0 if r < 1 else r * P
                        osl = slice(c0, 512)
                        kch = qk_sb[("k", h, i // 4)]
                        sps = ps.tile([P, 512], F32, tag=f"b{3 + (i + h) % 4}",
                                      name=f"s{h}_{nb}_{i}")
                        nc.tensor.matmul(
                            sps[:, osl],
                            kch[:, (i % 4) * P:(i % 4 + 1) * P],
                            qT[:, osl], start=True, stop=True)
                        pt = ptpool.tile([P, 512], F16, tag="pt", bufs=6)
                        nc.scalar.activation(pt[:, osl], sps[:, osl], EXP,
                                             scale=SCALE)
                        if r >= 0:
                            dsl = slice(r * P, (r + 1) * P)
                            nc.gpsimd.tensor_tensor(
                                pt[:, dsl], pt[:, dsl], tri_sb[:], op=MULT)
                        nc.tensor.matmul(
                            ctx_ps[:, osl],
                            v_sb[i][:, h * P:(h + 1) * P], pt[:, osl],
                            start=(i == 0), stop=(i == nt - 1))
                        if i == 0:
                            nc.vector.tensor_copy(lacc[:], pt[:])
                        else:
                            nc.vector.tensor_tensor(
                                lacc[:, osl], lacc[:, osl], pt[:, osl], op=ADD)
                        if h >= 1:
                            drain(per_step)
                    if h == HPC - 1:
                        if pending is not None:
                            stats(*pending)
                            pending = None
                        drain(6)
                        stats(h, ctx_ps, lacc)
                        drain(NKT * HPC)
                    else:
                        pending = (h, ctx_ps, lacc)
                if nb < NB - 1:
                    for h in range(HPC):
                        rope_one(nb + 1, "q", h)
                return ctx_tiles

            def outproj(nb, ctx_tiles):
                last = nb == NB - 1
                for sub in range(4):
                    tt = nb * 4 + sub
                    ssl = slice(sub * P, (sub + 1) * P)
                    ysb = ypool.tile([P, C], F16, tag="ysb", bufs=2)
                    for ob in range(NB):
                        obsl = slice(ob * 512, (ob + 1) * 512)
                        # 4-bank ring absorbs eviction latency; b1 frees
                        # earliest for the k-wave that follows
                        yps = ps.tile([P, 512], F32,
                                      tag=("b1", "b2", "b3", "b4")[ob],
                                      name=f"y{tt}_{ob}")
                        for h in range(HPC):
                            nc.tensor.matmul(
                                yps[:], ctx_tiles[h][:, ssl],
                                wo_sb[:, h, obsl],
                                start=(h == 0), stop=(h == HPC - 1))
                        if ob % 2 == 0:
                            nc.vector.tensor_copy(ysb[:, obsl], yps[:])
                        else:
                            nc.scalar.copy(ysb[:, obsl], yps[:])
                        if last:
                            nc.sync.dma_start(
                                y[tt * P:(tt + 1) * P, obsl], ysb[:, obsl])
                    if not last:
                        nc.sync.dma_start(y[tt * P:(tt + 1) * P, :], ysb[:])

            # ---- startup DMAs: x(0)+wq+wk interleaved, fine-grained head ----
            x0 = xpool.tile([P, NKT, 512], F16, tag="x", bufs=2, name="x_nb0")
            groups = [(0, 1), (1, 2), (2, 4), (4, 8), (8, 12), (12, 16)]
            for lo, hi in groups:
                gs = slice(lo, hi)
                nc.sync.dma_start(x0[:, gs, :], xg[:, gs, 0:512])
                nc.sync.dma_start(wq_sb[:, gs, :], wq[:, gs, :])
                nc.sync.dma_start(wk_sb[:, gs, :], wk[:, gs, :])
            nc.sync.dma_start(cos_sb[:], cosT)
            nc.sync.dma_start(sin_sb[:], sinT)
            nc.sync.dma_start(wv_sb[:], wv)
            nc.sync.dma_start(tri_sb[:], tri)
            nc.sync.dma_start(ones_sb[:], ones)
            nc.sync.dma_start(wo_sb[:], wo)

            def v_group(nbv, xt_src):
                """tt0 evicted immediately; tt1..3 stay in PSUM for the JIT
                evictions inside attention(nbv) head 0."""
                va0 = v_chain(nbv, 0, xt_src, "b1")
                nc.scalar.copy(v_sb[nbv * 4][:], va0[:])
                return {tt: v_chain(nbv, tt, xt_src, ("b1", "b2", "b7")[tt - 1])
                        for tt in (1, 2, 3)}

            # ---- block 0: plain qk (kt-outer tracks DMA arrival), v ----
            qaccs = wave_accs(QBANKS, "q")
            kaccs = wave_accs(KBANKS, "k")
            for kt in range(NKT):
                for part, wsb, accs in (("q", wq_sb, qaccs),
                                        ("k", wk_sb, kaccs)):
                    for h in range(HPC):
                        nc.tensor.matmul(
                            accs[h][:], wsb[:, kt, h * P:(h + 1) * P],
                            x0[:, kt, :], start=(kt == 0), stop=(kt == NKT - 1))
            for h in range(HPC):
                evict_rope_one(0, "k", h, kaccs[h])
            for h in range(HPC):
                evict_rope_one(0, "q", h, qaccs[h])
            vaccs = v_group(0, x0)

            xt = x0
            for nb in range(NB):
                # prefetch next block's x during attention
                if nb < NB - 1:
                    xt_next = xpool.tile([P, NKT, 512], F16, tag="x", bufs=2,
                                         name=f"x_nb{nb + 1}")
                    nsl2 = slice((nb + 1) * 512, (nb + 2) * 512)
                    # halves: the first q-filler chains only need low kt
                    nc.sync.dma_start(xt_next[:, 0:8, :], xg[:, 0:8, nsl2])
                    nc.sync.dma_start(xt_next[:, 8:16, :], xg[:, 8:16, nsl2])
                else:
                    xt_next = xt
                ctx_tiles = attention(nb, xt_next, vaccs)
                outproj(nb, ctx_tiles)
                if nb < NB - 1:
                    for h in range(HPC):
                        ka = qk_wave_chain("k", xt_next, h, f"b{1 + h % 2}")
                        evict_rope_one(nb + 1, "k", h, ka)
                    if nb + 1 < NB - 1:
                        vaccs = v_group(nb + 1, xt_next)
                    else:
                        va0 = v_chain(nb + 1, 0, xt_next, "b1")
                        nc.scalar.copy(v_sb[(nb + 1) * 4][:], va0[:])
                        vaccs = {}
                xt = xt_next

    nc.compile()
    return nc


def _build_kernel():
    if "k" not in _CACHE:
        _CACHE["k"] = _build()
    return _CACHE["k"]


def prepare_in_maps(x, W_qkv, W_o, cos, sin):
    f16 = np.float16
    tri01 = (np.arange(P)[:, None] <= np.arange(P)[None, :]).astype(f16)
    ones = np.ones((P, P), dtype=f16)
    cosT = np.ascontiguousarray(cos.T).astype(f16)
    # rotate_half sign folded in: rows (head dims) 0..63 negated
    sgn = np.where(np.arange(P) < P // 2, -1.0, 1.0).astype(np.float32)
    sinT = (sin.T * sgn[:, None]).astype(f16)

    in_maps = []
    for core in range(NCORES):
        b = core // 4
        hg0 = (core % 4) * HPC
        rows = slice(hg0 * P, (hg0 + HPC) * P)
        xT = x[b].T  # [C, T]
        xg = np.ascontiguousarray(
            xT.reshape(NKT, P, T).transpose(1, 0, 2)).astype(f16)

        def wprep(w):  # [512 rows, 2048 c] -> [P, NKT, 512]
            return np.ascontiguousarray(
                w.T.reshape(NKT, P, HPC * P).transpose(1, 0, 2)).astype(f16)

        wq_t = wprep(W_qkv[0 * C:1 * C][rows])
        wk_t = wprep(W_qkv[1 * C:2 * C][rows])
        wv_t = wprep(W_qkv[2 * C:3 * C][rows])
        wo_t = np.ascontiguousarray(
            W_o[:, rows].T.reshape(HPC, P, C).transpose(1, 0, 2)).astype(f16)
        in_maps.append({
            "xg": xg, "wq": wq_t, "wk": wk_t, "wv": wv_t, "wo": wo_t,
            "cosT": cosT, "sinT": sinT, "tri": tri01, "ones": ones,
        })
    return in_maps


def gather(results, b_o):
    y = np.zeros((2, T, C), dtype=np.float32)
    for core in range(NCORES):
        y[core // 4] += results[core]["y"].astype(np.float32)
    y += np.asarray(b_o, dtype=np.float32)[None, None, :]
    return y


def kernel(x, W_qkv, W_o, b_o, cos, sin):
    x = np.asarray(x, dtype=np.float32)
    W_qkv = np.asarray(W_qkv, dtype=np.float32)
    W_o = np.asarray(W_o, dtype=np.float32)
    cos = np.asarray(cos, dtype=np.float32)
    sin = np.asarray(sin, dtype=np.float32)
    nc = _build_kernel()
    in_maps = prepare_in_maps(x, W_qkv, W_o, cos, sin)
    res = run_bass_kernel_spmd(nc, in_maps, core_ids=list(range(NCORES)))
    return gather(res.results, b_o)


# revision 22
# speedup vs baseline: 1.3131x; 1.0156x over previous
"""Causal multi-head attention (RoPE) on 8 TRN2 NeuronCores.

Problem: x[2,2048,2048] -> qkv proj -> rope -> causal attention (16 heads,
head_dim 128) -> output proj + bias. Sharding: (batch, head-group) across the
8 cores - core c handles batch c//4 and heads 4*(c%4)..4*(c%4)+3. Each core
computes a partial output projection over its heads' channels; the host sums
the 4 partials per batch and adds b_o.

Single-pass token-outer pipeline, everything fp16 on device (PSUM accumulation
stays f32; final host reduction in f32; validated rel err ~5e-4 vs the fp32
reference). The exp throughput on ACT (0.833ns/col) exactly matches the
scores+AV cost on PE, so attention phases are ACT-bound unless PE borrows
other work: the next block's QKV projection is software-pipelined INTO the
attention window as three waves:

    attn(nb) heads -> q-wave(nb+1) -> outproj(nb) -> k-wave(nb+1)
                   -> v-wave(nb+1) -> attn(nb+1) ...

Waves are accumulator-major (16 kt matmuls per PSUM bank) with a bank map
chosen so each wave's first banks were freed earliest by the previous phase:
q-wave on b1,b2,b0,b7 / k-wave on b3..b6 / v-wave on b0,b7,b1,b2; attention
rotates scores over b3..b6 4-deep (tag b{3+(i+h)%4}), softmax-denominator
broadcast lb takes the next slot in that rotation, ctx alternates b0/b7.

Scores are transposed s^T[tk,tq] (lhsT=k tile, rhs=q block) with causal
narrowing; matmul cost here is (moving columns) x (cycles/row keyed on the
MOVING operand dtype): fp16 runs 1 cycle/row with no 256-column minimum, so
the r=3 diagonal tile narrows to 128 columns. Softmax denominators come from
element-wise fp16 accumulation of the exp tiles on DVE (2x mode) + ONE
ones-matmul per (head, block) that broadcasts the partition sum - the
per-tile [1,512] ones-matmuls this replaces cost a full 30us of PE. RoPE is
applied in place (half-swap via 2 small SBUF DMAs, sign folded into sinT on
the host; mults split Pool/DVE). Output projection accumulates the 4 heads
in PSUM per 128-token sub-tile; quarter evictions alternate DVE/ACT into an
fp16 [128,2048] staging row, one DMA per sub-tile (per-quarter DMAs on the
last block to shorten the tail). DMAs are batched multi-kt loads; x for block
nb+1 prefetches during attention nb.
"""
import math

import numpy as np

import concourse.bacc as bacc
import concourse.mybir as mybir
import concourse.tile as tile
from concourse.bass_utils import run_bass_kernel_spmd

P = 128           # partitions / head_dim
T = 2048          # context length
C = 2048          # d_model
NKT = C // P      # 16 contraction tiles
NB = T // 512     # 4 token blocks of 512
HPC = 4           # heads per core
NCORES = 8
SCALE = 1.0 / math.sqrt(P)

F32 = mybir.dt.float32
F16 = mybir.dt.float16
EXP = mybir.ActivationFunctionType.Exp
MULT = mybir.AluOpType.mult
ADD = mybir.AluOpType.add

QBANKS = ("b1", "b2", "b0", "b7")   # q-wave accumulators, emission order
KBANKS = ("b3", "b4", "b5", "b6")   # k-wave accumulators
VBANKS = ("b0", "b7", "b1", "b2")   # v-wave accumulators
CTXBANKS = ("b0", "b7")             # ctx_ps alternates by head parity

_CACHE = {}


def _build():
    nc = bacc.Bacc("TRN2", target_bir_lowering=False, debug=False,
                   num_devices=NCORES)
    xg = nc.dram_tensor("xg", (P, NKT, T), F16, kind="ExternalInput").ap()
    wq = nc.dram_tensor("wq", (P, NKT, HPC * P), F16, kind="ExternalInput").ap()
    wk = nc.dram_tensor("wk", (P, NKT, HPC * P), F16, kind="ExternalInput").ap()
    wv = nc.dram_tensor("wv", (P, NKT, HPC * P), F16, kind="ExternalInput").ap()
    wo = nc.dram_tensor("wo", (P, HPC, C), F16, kind="ExternalInput").ap()
    cosT = nc.dram_tensor("cosT", (P, T), F16, kind="ExternalInput").ap()
    sinT = nc.dram_tensor("sinT", (P, T), F16, kind="ExternalInput").ap()
    tri = nc.dram_tensor("tri", (P, P), F16, kind="ExternalInput").ap()
    ones = nc.dram_tensor("ones", (P, P), F16, kind="ExternalInput").ap()
    y = nc.dram_tensor("y", (T, C), F16, kind="ExternalOutput").ap()

    half = P // 2

    with tile.TileContext(nc) as tc:
        with (
            tc.tile_pool(name="gconst", bufs=1) as gpool,
            tc.tile_pool(name="wbuf", bufs=1) as wpool,
            tc.tile_pool(name="xbuf", bufs=1) as xpool,
            tc.tile_pool(name="qkbuf", bufs=1) as qkpool,
            tc.tile_pool(name="vbuf", bufs=1) as vpool,
            tc.tile_pool(name="rope", bufs=1) as rpool,
            tc.tile_pool(name="ptb", bufs=1) as ptpool,
            tc.tile_pool(name="stats", bufs=1) as spool,
            tc.tile_pool(name="ctxb", bufs=1) as cxpool,
            tc.tile_pool(name="yb", bufs=1) as ypool,
            tc.tile_pool(name="ps", bufs=1, space="PSUM") as ps,
        ):
            tri_sb = gpool.tile([P, P], F16, tag="tri")
            ones_sb = gpool.tile([P, P], F16, tag="ones")
            wq_sb = wpool.tile([P, NKT, HPC * P], F16, tag="wq", name="wq_sb")
            wk_sb = wpool.tile([P, NKT, HPC * P], F16, tag="wk", name="wk_sb")
            wv_sb = wpool.tile([P, NKT, HPC * P], F16, tag="wv", name="wv_sb")
            wo_sb = wpool.tile([P, HPC, C], F16, tag="wo", name="wo_sb")
            cos_sb = wpool.tile([P, T], F16, tag="cos", name="cos_sb")
            sin_sb = wpool.tile([P, T], F16, tag="sin", name="sin_sb")

            qk_sb = {}
            for h in range(HPC):
                for part in ("q", "k"):
                    for nb in range(NB):
                        qk_sb[(part, h, nb)] = qkpool.tile(
                            [P, 512], F16, tag=f"{part}{h}n{nb}",
                            name=f"{part}{h}n{nb}_sb")
            v_sb = [vpool.tile([P, 512], F16, tag=f"vb{i}", name=f"v{i}_sb")
                    for i in range(NKT)]

            def wave_accs(banks, label):
                return [ps.tile([P, 512], F32, tag=banks[h],
                                name=f"{label}{h}") for h in range(HPC)]

            def rope_one(nb, part, h):
                """In-place rope on an evicted q/k chunk. Emitted away from
                the attention masks: the Pool t1 multiply is 1.1us, and a
                diagonal mask queued behind it stalls the AV matmuls."""
                nsl = slice(nb * 512, (nb + 1) * 512)
                dst = qk_sb[(part, h, nb)]
                tmp = rpool.tile([P, 512], F16, tag="rt", bufs=2, name="rtmp")
                nc.sync.dma_start(tmp[0:half, :], dst[half:P, :])
                nc.sync.dma_start(tmp[half:P, :], dst[0:half, :])
                t1 = rpool.tile([P, 512], F16, tag="t1", bufs=2)
                nc.gpsimd.tensor_tensor(t1[:], dst[:], cos_sb[:, nsl], op=MULT)
                t2 = rpool.tile([P, 512], F16, tag="t2", bufs=2)
                nc.vector.tensor_tensor(t2[:], tmp[:], sin_sb[:, nsl], op=MULT)
                nc.vector.tensor_tensor(dst[:], t1[:], t2[:], op=ADD)

            def evict_rope_one(nb, part, h, acc):
                nc.scalar.copy(qk_sb[(part, h, nb)][:], acc[:])
                rope_one(nb, part, h)

            def qk_wave_chain(part, xt, h, bank):
                wsb = wq_sb if part == "q" else wk_sb
                acc = ps.tile([P, 512], F32, tag=bank, name=f"{part}{h}")
                for kt in range(NKT):
                    nc.tensor.matmul(
                        acc[:], wsb[:, kt, h * P:(h + 1) * P],
                        xt[:, kt, :], start=(kt == 0), stop=(kt == NKT - 1))
                return acc

            def v_chain(nb, tt, xt, bank):
                vacc = ps.tile([P, 512], F32, tag=bank, name=f"v{nb}_{tt}")
                for kt in range(NKT):
                    nc.tensor.matmul(
                        vacc[:], xt[:, kt, tt * P:(tt + 1) * P],
                        wv_sb[:, kt, :], start=(kt == 0), stop=(kt == NKT - 1))
                return vacc

            def attention(nb, xt_next, vaccs):
                """vaccs: this block's un-evicted v accumulators (tt 1..3 on
                b1,b2,b7); evictions are emitted just-in-time at the diagonal
                steps of head 0 so ACT serves head 0's first exps first. For
                nb==3 the tt>=1 v chains are emitted inside head 0 as PE
                filler (no next-block waves exist). Head h's softmax stats
                (lb matmul, reciprocal, normalize) are deferred into head
                h+1's pipeline so PE never waits on the DVE denominator
                chain. For nb<3 the next block's q-wave chain for head h is
                emitted right after head h (banks b1/b2 alternating, evicted
                and roped immediately)."""
                nt = 4 * (nb + 1)
                ctx_tiles = {}
                pending = None

                def q_filler():
                    # next block's q projection, one matmul per drain unit,
                    # each chain evicted (ACT) as soon as it completes
                    for fh in range(HPC):
                        acc = ps.tile([P, 512], F32, tag=f"b{1 + fh % 2}",
                                      name=f"q{fh}")
                        for kt in range(NKT):
                            nc.tensor.matmul(
                                acc[:], wq_sb[:, kt, fh * P:(fh + 1) * P],
                                xt_next[:, kt, :], start=(kt == 0),
                                stop=(kt == NKT - 1))
                            yield
                        nc.scalar.copy(qk_sb[("q", fh, nb + 1)][:], acc[:])

                filler = q_filler() if nb < NB - 1 else None

                def drain(n):
                    if filler is None:
                        return
                    for _ in range(n):
                        if next(filler, "done") == "done":
                            break

                per_step = max(1, 58 // (3 * nt))

                def stats(h, ctx_ps, lacc):
                    # slot (h+3)%4 is the one head h+1 touches last after the
                    # deferred emission point (its i==3), so the reciprocal
                    # drains before the bank is needed again
                    lb = ps.tile([P, 512], F32, tag=f"b{3 + (h + 3) % 4}",
                                 name=f"l{h}_{nb}")
                    nc.tensor.matmul(lb[:], ones_sb[:], lacc[:],
                                     start=True, stop=True)
                    rinv = spool.tile([P, 512], F32, tag="rinv", bufs=2)
                    ctx_sb = cxpool.tile([P, 512], F16, tag=f"cx{h}", bufs=2,
                                         name=f"cs{h}_{nb}")
                    # last head's normalize gates the output projection: do it
                    # in chunks so outproj's first sub-tile unblocks early
                    for lo, hi in ((0, 128), (128, 256), (256, 512)) \
                            if h == HPC - 1 else ((0, 512),):
                        csl = slice(lo, hi)
                        nc.vector.reciprocal(rinv[:, csl], lb[:, csl])
                        nc.vector.tensor_tensor(ctx_sb[:, csl],
                                                ctx_ps[:, csl],
                                                rinv[:, csl], op=MULT)
                    ctx_tiles[h] = ctx_sb

                for h in range(HPC):
                    qT = qk_sb[("q", h, nb)]
                    ctx_ps = ps.tile([P, 512], F32, tag=CTXBANKS[h % 2],
                                     name=f"ctx{h}_{nb}")
                    lacc = spool.tile([P, 512], F16, tag="lacc", bufs=2)
                    for i in range(nt):
                        r = i - 4 * nb
                        if h == 0 and nb == NB - 1 and r in (-9, -6, -3):
                            # pull this block's remaining v chains in as filler
                            tt = (r + 9) // 3 + 1
                            vaccs[tt] = v_chain(nb, tt, xt_next,
                                                ("b1", "b2", "b7")[tt - 1])
                        if h == 0 and r >= 1:
                            if nb == NB - 1:
                                # keep ACT free for the exp stream in the
                                # last (largest, ACT-bound) block
                                nc.vector.tensor_copy(v_sb[nb * 4 + r][:],
                                                      vaccs[r][:])
                            else:
                                nc.scalar.copy(v_sb[nb * 4 + r][:],
                                               vaccs[r][:])
                        if h > 0 and i == 3 and pending is not None:
                            stats(*pending)
                            pending = None
                        c0 = 0 if r < 1 else r * P
                        osl = slice(c0, 512)
                        kch = qk_sb[("k", h, i // 4)]
                        sps = ps.tile([P, 512], F32, tag=f"b{3 + (i + h) % 4}",
                                      name=f"s{h}_{nb}_{i}")
                        nc.tensor.matmul(
                            sps[:, osl],
                            kch[:, (i % 4) * P:(i % 4 + 1) * P],
                            qT[:, osl], start=True, stop=True)
                        pt = ptpool.tile([P, 512], F16, tag="pt", bufs=6)
                        nc.scalar.activation(pt[:, osl], sps[:, osl], EXP,
                                             scale=SCALE)
                        if r >= 0:
                            # diagonal mask on DVE: fp16 2x mode takes 127ns
                            # vs Pool's 444ns+launch, and it's on the exp->AV
                            # critical path every diagonal step
                            dsl = slice(r * P, (r + 1) * P)
                            nc.vector.tensor_tensor(
                                pt[:, dsl], pt[:, dsl], tri_sb[:], op=MULT)
                        nc.tensor.matmul(
                            ctx_ps[:, osl],
                            v_sb[i][:, h * P:(h + 1) * P], pt[:, osl],
                            start=(i == 0), stop=(i == nt - 1))
                        if i == 0:
                            nc.vector.tensor_copy(lacc[:], pt[:])
                        else:
                            nc.vector.tensor_tensor(
                                lacc[:, osl], lacc[:, osl], pt[:, osl], op=ADD)
                        if h >= 1:
                            drain(per_step)
                    if h == HPC - 1:
                        if pending is not None:
                            stats(*pending)
                            pending = None
                        drain(6)
                        stats(h, ctx_ps, lacc)
                        drain(NKT * HPC)
                    else:
                        pending = (h, ctx_ps, lacc)
                if nb < NB - 1:
                    for h in range(HPC):
                        rope_one(nb + 1, "q", h)
                return ctx_tiles

            def outproj(nb, ctx_tiles):
                last = nb == NB - 1
                for sub in range(4):
                    tt = nb * 4 + sub
                    ssl = slice(sub * P, (sub + 1) * P)
                    ysb = ypool.tile([P, C], F16, tag="ysb", bufs=2)
                    ypss = {}
                    if last and sub == 0:
                        # on the final block the h3 normalize is still in
                        # flight; open all 4 accumulation groups with their
                        # h0..h2 contributions first to cover its latency
                        for ob in range(NB):
                            obsl = slice(ob * 512, (ob + 1) * 512)
                            yps = ypss[ob] = ps.tile(
                                [P, 512], F32,
                                tag=("b1", "b2", "b3", "b4")[ob],
                                name=f"y{tt}_{ob}")
                            for h in range(HPC - 1):
                                nc.tensor.matmul(
                                    yps[:], ctx_tiles[h][:, ssl],
                                    wo_sb[:, h, obsl],
                                    start=(h == 0), stop=False)
                    for ob in range(NB):
                        obsl = slice(ob * 512, (ob + 1) * 512)
                        # 4-bank ring absorbs eviction latency; b1 frees
                        # earliest for the k-wave that follows
                        if ob in ypss:
                            yps = ypss[ob]
                            nc.tensor.matmul(
                                yps[:], ctx_tiles[HPC - 1][:, ssl],
                                wo_sb[:, HPC - 1, obsl],
                                start=False, stop=True)
                        else:
                            yps = ps.tile([P, 512], F32,
                                          tag=("b1", "b2", "b3", "b4")[ob],
                                          name=f"y{tt}_{ob}")
                            halves = ((0, 512),)
                            if last and sub == 3 and ob == 3:
                                # split the final group so its first half's
                                # eviction/DMA overlaps the second half
                                halves = ((0, 256), (256, 512))
                            for lo, hi in halves:
                                for h in range(HPC):
                                    nc.tensor.matmul(
                                        yps[:, lo:hi], ctx_tiles[h][:, ssl],
                                        wo_sb[:, h, ob * 512 + lo:
                                              ob * 512 + hi],
                                        start=(h == 0), stop=(h == HPC - 1))
                                if hi - lo < 512:
                                    nc.vector.tensor_copy(
                                        ysb[:, ob * 512 + lo:ob * 512 + hi],
                                        yps[:, lo:hi])
                                    nc.sync.dma_start(
                                        y[tt * P:(tt + 1) * P,
                                          ob * 512 + lo:ob * 512 + hi],
                                        ysb[:, ob * 512 + lo:ob * 512 + hi])
                        if last and sub == 3 and ob == 3:
                            continue
                        if ob % 2 == 0:
                            nc.vector.tensor_copy(ysb[:, obsl], yps[:])
                        else:
                            nc.scalar.copy(ysb[:, obsl], yps[:])
                        if last:
                            nc.sync.dma_start(
                                y[tt * P:(tt + 1) * P, obsl], ysb[:, obsl])
                    if not last:
                        nc.sync.dma_start(y[tt * P:(tt + 1) * P, :], ysb[:])

            # ---- startup DMAs: x(0)+wq+wk interleaved, fine-grained head ----
            x0 = xpool.tile([P, NKT, 512], F16, tag="x", bufs=2, name="x_nb0")
            groups = [(0, 1), (1, 2), (2, 4), (4, 8), (8, 12), (12, 16)]
            for lo, hi in groups:
                gs = slice(lo, hi)
                nc.sync.dma_start(x0[:, gs, :], xg[:, gs, 0:512])
                nc.sync.dma_start(wq_sb[:, gs, :], wq[:, gs, :])
                nc.sync.dma_start(wk_sb[:, gs, :], wk[:, gs, :])
            nc.sync.dma_start(cos_sb[:], cosT)
            nc.sync.dma_start(sin_sb[:], sinT)
            nc.sync.dma_start(wv_sb[:], wv)
            nc.sync.dma_start(tri_sb[:], tri)
            nc.sync.dma_start(ones_sb[:], ones)
            nc.sync.dma_start(wo_sb[:], wo)

            def v_group(nbv, xt_src):
                """tt0 evicted immediately; tt1..3 stay in PSUM for the JIT
                evictions inside attention(nbv) head 0."""
                va0 = v_chain(nbv, 0, xt_src, "b1")
                nc.scalar.copy(v_sb[nbv * 4][:], va0[:])
                return {tt: v_chain(nbv, tt, xt_src, ("b1", "b2", "b7")[tt - 1])
                        for tt in (1, 2, 3)}

            # ---- block 0: plain qk (kt-outer tracks DMA arrival), v ----
            qaccs = wave_accs(QBANKS, "q")
            kaccs = wave_accs(KBANKS, "k")
            for kt in range(NKT):
                for part, wsb, accs in (("q", wq_sb, qaccs),
                                        ("k", wk_sb, kaccs)):
                    for h in range(HPC):
                        nc.tensor.matmul(
                            accs[h][:], wsb[:, kt, h * P:(h + 1) * P],
                            x0[:, kt, :], start=(kt == 0), stop=(kt == NKT - 1))
            for h in range(HPC):
                evict_rope_one(0, "k", h, kaccs[h])
            for h in range(HPC):
                evict_rope_one(0, "q", h, qaccs[h])
            vaccs = v_group(0, x0)

            xt = x0
            for nb in range(NB):
                # prefetch next block's x during attention
                if nb < NB - 1:
                    xt_next = xpool.tile([P, NKT, 512], F16, tag="x", bufs=2,
                                         name=f"x_nb{nb + 1}")
                    nsl2 = slice((nb + 1) * 512, (nb + 2) * 512)
                    # halves: the first q-filler chains only need low kt
                    nc.sync.dma_start(xt_next[:, 0:8, :], xg[:, 0:8, nsl2])
                    nc.sync.dma_start(xt_next[:, 8:16, :], xg[:, 8:16, nsl2])
                else:
                    xt_next = xt
                ctx_tiles = attention(nb, xt_next, vaccs)
                outproj(nb, ctx_tiles)
                if nb < NB - 1:
                    for h in range(HPC):
                        ka = qk_wave_chain("k", xt_next, h, f"b{1 + h % 2}")
                        evict_rope_one(nb + 1, "k", h, ka)
                    if nb + 1 < NB - 1:
                        vaccs = v_group(nb + 1, xt_next)
                    else:
                        va0 = v_chain(nb + 1, 0, xt_next, "b1")
                        nc.scalar.copy(v_sb[(nb + 1) * 4][:], va0[:])
                        vaccs = {}
                xt = xt_next

    nc.compile()
    return nc


def _build_kernel():
    if "k" not in _CACHE:
        _CACHE["k"] = _build()
    return _CACHE["k"]


def prepare_in_maps(x, W_qkv, W_o, cos, sin):
    f16 = np.float16
    tri01 = (np.arange(P)[:, None] <= np.arange(P)[None, :]).astype(f16)
    ones = np.ones((P, P), dtype=f16)
    cosT = np.ascontiguousarray(cos.T).astype(f16)
    # rotate_half sign folded in: rows (head dims) 0..63 negated
    sgn = np.where(np.arange(P) < P // 2, -1.0, 1.0).astype(np.float32)
    sinT = (sin.T * sgn[:, None]).astype(f16)

    in_maps = []
    for core in range(NCORES):
        b = core // 4
        hg0 = (core % 4) * HPC
        rows = slice(hg0 * P, (hg0 + HPC) * P)
        xT = x[b].T  # [C, T]
        xg = np.ascontiguousarray(
            xT.reshape(NKT, P, T).transpose(1, 0, 2)).astype(f16)

        def wprep(w):  # [512 rows, 2048 c] -> [P, NKT, 512]
            return np.ascontiguousarray(
                w.T.reshape(NKT, P, HPC * P).transpose(1, 0, 2)).astype(f16)

        wq_t = wprep(W_qkv[0 * C:1 * C][rows])
        wk_t = wprep(W_qkv[1 * C:2 * C][rows])
        wv_t = wprep(W_qkv[2 * C:3 * C][rows])
        wo_t = np.ascontiguousarray(
            W_o[:, rows].T.reshape(HPC, P, C).transpose(1, 0, 2)).astype(f16)
        in_maps.append({
            "xg": xg, "wq": wq_t, "wk": wk_t, "wv": wv_t, "wo": wo_t,
            "cosT": cosT, "sinT": sinT, "tri": tri01, "ones": ones,
        })
    return in_maps


def gather(results, b_o):
    y = np.zeros((2, T, C), dtype=np.float32)
    for core in range(NCORES):
        y[core // 4] += results[core]["y"].astype(np.float32)
    y += np.asarray(b_o, dtype=np.float32)[None, None, :]
    return y


def kernel(x, W_qkv, W_o, b_o, cos, sin):
    x = np.asarray(x, dtype=np.float32)
    W_qkv = np.asarray(W_qkv, dtype=np.float32)
    W_o = np.asarray(W_o, dtype=np.float32)
    cos = np.asarray(cos, dtype=np.float32)
    sin = np.asarray(sin, dtype=np.float32)
    nc = _build_kernel()
    in_maps = prepare_in_maps(x, W_qkv, W_o, cos, sin)
    res = run_bass_kernel_spmd(nc, in_maps, core_ids=list(range(NCORES)))
    return gather(res.results, b_o)
